# revision 1
# baseline (speedup 1.0000x reference)
"""Single-head attention (B=4, S=2048, D=E=1024) on 8 trn2 NeuronCores.

Sharding: data-parallel over (batch, q-half) -> 8 shards. Each core gets a
1024-row q shard plus the full 2048 keys of its batch; K/V projections are
recomputed on both cores of a batch pair (25% extra flops, zero collectives).

Per-core math (all "T" tensors are token-transposed on the host so that the
contraction dim lands on SBUF partitions; no on-device transposes needed):
  qp^T [E,q]   = (lhsT=wq[D,E], rhs=qT[D,q]) * (1/sqrt E) + bq/sqrt(E)
  kp^T [E,k]   = (lhsT=wk, rhs=kT) + bk
  vp   [k,E]   = (lhsT=vT[D,k], rhs=wv[D,E]) + bv
  lgT  [k,q]   = (lhsT=kp^T slice, rhs=qp^T)            (scale folded into qp)
  expT [k,q]   = Exp(lgT + mask*NEG)                    (ACT, per-partition bias)
  s    [.,q]   = ones-matmul over expT                  (softmax sum; no max-sub:
                                                         logits ~ N(0,1), safe)
  ctx^T[E,q]   = (lhsT=vp slice, rhs=expT) * recip(s)
  out  [q,D]   = (lhsT=ctx^T slice, rhs=ow[E,D]) + ob
All matmuls run as float32r (full PE rate at N>=256), fp32 data + accumulate.
Pool lifetimes follow strict LIFO (Tile pool-stack requirement).
"""

import os
import numpy as np

P = 128
NEG = -1.0e9


def build_nc(D=1024, E=1024, SK=2048, QSH=1024, QB=512):
    """Build the per-core Bass module (SPMD; same program on all cores)."""
    import concourse.bass as bass
    import concourse.mybir as mybir
    import concourse.tile as tile
    from concourse import bacc

    f32 = mybir.dt.float32
    f32r = mybir.dt.float32r
    AF = mybir.ActivationFunctionType

    DT = D // P          # contraction tiles over model dim
    ET = E // P          # enc tiles
    KT = SK // P         # key tiles
    NQB = QSH // QB      # q blocks
    KNB = min(512, SK)   # key free-dim block for kp
    ENB = min(512, E)    # E free-dim block for vp
    DNB = min(512, D)    # model free-dim block for out
    DTH = max(1, DT // 2)  # split-K half for kp streaming
    ISCALE = 1.0 / float(np.sqrt(E))

    nc = bacc.Bacc(trn_type="TRN2")

    # ---- I/O ----
    qT = nc.dram_tensor("qT", [D, QSH], f32r, kind="ExternalInput")[:, :]
    kT = nc.dram_tensor("kT", [D, SK], f32r, kind="ExternalInput")[:, :]
    vT = nc.dram_tensor("vT", [D, SK], f32r, kind="ExternalInput")[:, :]
    mask_cols = nc.dram_tensor("mask_cols", [P, KT], f32, kind="ExternalInput")[:, :]
    ones_d = nc.dram_tensor("ones_d", [P, P], f32r, kind="ExternalInput")[:, :]
    wq = nc.dram_tensor("wq", [D, E], f32r, kind="ExternalInput")[:, :]
    wk = nc.dram_tensor("wk", [D, E], f32r, kind="ExternalInput")[:, :]
    wv = nc.dram_tensor("wv", [D, E], f32r, kind="ExternalInput")[:, :]
    ow = nc.dram_tensor("ow", [E, D], f32r, kind="ExternalInput")[:, :]
    bq_col = nc.dram_tensor("bq_col", [P, ET], f32, kind="ExternalInput")[:, :]
    bk_col = nc.dram_tensor("bk_col", [P, ET], f32, kind="ExternalInput")[:, :]
    bv_bc = nc.dram_tensor("bv_bc", [P, E], f32, kind="ExternalInput")[:, :]
    ob_bc = nc.dram_tensor("ob_bc", [P, D], f32, kind="ExternalInput")[:, :]
    out = nc.dram_tensor("out", [QSH, D], f32, kind="ExternalOutput")[:, :]

    qT_r = qT.rearrange("(t p) n -> p t n", p=P)   # [128, DT, QSH]
    kT_r = kT.rearrange("(t p) n -> p t n", p=P)
    vT_r = vT.rearrange("(t p) n -> p t n", p=P)
    wq_r = wq.rearrange("(t p) n -> p t n", p=P)   # [128, DT, E]
    wk_r = wk.rearrange("(t p) n -> p t n", p=P)
    wv_r = wv.rearrange("(t p) n -> p t n", p=P)
    ow_r = ow.rearrange("(t p) n -> p t n", p=P)   # [128, ET, D]

    def mm(ps, lhsT, rhs, start, stop):
        nc.tensor.matmul(ps, lhsT, rhs, start=start, stop=stop)

    with tile.TileContext(nc) as tc:
        # ---- persistent smalls (incl. per-qb softmax reciprocal + out bias) ----
        smalls_cm = tc.tile_pool(name="smalls", bufs=1)
        smalls = smalls_cm.__enter__()
        ones_t = smalls.tile([P, P], f32r, name="ones")
        nc.gpsimd.dma_start(ones_t[:], ones_d)
        mask_t = smalls.tile([P, KT], f32, name="maskc")
        nc.gpsimd.dma_start(mask_t[:], mask_cols)
        nc.scalar.mul(mask_t[:], mask_t[:], NEG)
        bq_t = smalls.tile([P, ET], f32, name="bqc")
        nc.gpsimd.dma_start(bq_t[:], bq_col)
        nc.scalar.mul(bq_t[:], bq_t[:], ISCALE)
        bk_t = smalls.tile([P, ET], f32, name="bkc")
        nc.gpsimd.dma_start(bk_t[:], bk_col)
        recip_ts = [smalls.tile([P, QB], f32, name=f"recip{i}")
                    for i in range(NQB)]

        dram_cm = tc.tile_pool(name="dramscratch", bufs=1, space="DRAM")
        dram_pool = dram_cm.__enter__()

        # ---- phase VP (first: vp outlives kp): vp [SK, E] + bv ----
        vp_cm = tc.tile_pool(name="vp", bufs=1)
        vp_pool = vp_cm.__enter__()
        vp = vp_pool.tile([P, KT, E], f32r, name="vp")
        with tc.tile_pool(name="vp_w", bufs=1) as phw, \
             tc.tile_pool(name="vp_ph", bufs=3) as ph, \
             tc.tile_pool(name="vp_ps", bufs=4, space="PSUM") as php:
            wv_t = phw.tile([P, DT, E], f32r, name="wv_t")
            NH = 2 if E >= 512 else 1
            for h in range(NH):
                for t in range(DT):
                    nc.sync.dma_start(wv_t[:, t, h * E // NH:(h + 1) * E // NH],
                                      wv_r[:, t, h * E // NH:(h + 1) * E // NH])
            bv_t = phw.tile([P, E], f32, name="bv_t")
            nc.sync.dma_start(bv_t[:], bv_bc)
            for m in range(KT):
                lhs_t = ph.tile([P, DT, P], f32r, tag="vT_s", name=f"vT_{m}")
                hh = max(1, DT // 2)
                nc.scalar.dma_start(lhs_t[:, :hh, :],
                                    vT_r[:, :hh, m * P:(m + 1) * P])
                nc.gpsimd.dma_start(lhs_t[:, hh:, :],
                                    vT_r[:, hh:, m * P:(m + 1) * P])
                for n in range(E // ENB):
                    ps = php.tile([P, ENB], f32, tag="ps", name=f"vpps_{m}_{n}")
                    for t in range(DT):
                        mm(ps[:], lhs_t[:, t, :],
                           wv_t[:, t, n * ENB:(n + 1) * ENB],
                           t == 0, t == DT - 1)
                    nc.vector.tensor_add(vp[:, m, n * ENB:(n + 1) * ENB], ps[:],
                                         bv_t[:, n * ENB:(n + 1) * ENB])

        # ---- phase KP: kp^T [E, SK] + bk (kT streamed in split-K halves) ----
        kp_cm = tc.tile_pool(name="kp", bufs=1)
        kp_pool = kp_cm.__enter__()
        kp = kp_pool.tile([P, ET, SK], f32r, name="kp")
        with tc.tile_pool(name="kp_w", bufs=1) as phw, \
             tc.tile_pool(name="kp_ph", bufs=3) as ph, \
             tc.tile_pool(name="kp_ps", bufs=1, space="PSUM") as php:
            wk_t = phw.tile([P, DT, E], f32r, name="wk_t")
            for h in range(2):
                for t in range(DT):
                    eng = nc.sync if t % 2 == 0 else nc.scalar
                    eng.dma_start(wk_t[:, t, h * E // 2:(h + 1) * E // 2],
                                  wk_r[:, t, h * E // 2:(h + 1) * E // 2])
            for n in range(SK // KNB):
                pss = [php.tile([P, KNB], f32, tag=f"ps{m}", name=f"kpps_{n}_{m}")
                       for m in range(ET)]
                for th in range(DT // DTH):
                    rhs_t = ph.tile([P, DTH, KNB], f32r, tag="kT_s",
                                    name=f"kT_{n}_{th}")
                    for ti in range(DTH):
                        t = th * DTH + ti
                        eng = nc.gpsimd
                        eng.dma_start(rhs_t[:, ti, :],
                                      kT_r[:, t, n * KNB:(n + 1) * KNB])
                    for m in range(ET):
                        for ti in range(DTH):
                            t = th * DTH + ti
                            mm(pss[m][:], wk_t[:, t, m * P:(m + 1) * P],
                               rhs_t[:, ti, :], t == 0, t == DT - 1)
                for m in range(ET):
                    nc.scalar.activation(kp[:, m, n * KNB:(n + 1) * KNB],
                                         pss[m][:], AF.Identity,
                                         bias=bk_t[:, m:m + 1])

        # ---- attention per q-block ----
        ctx_bounce = []
        ctx_last = None
        ctx_last_cm = None

        for qb in range(NQB):
            q0 = qb * QB
            last_qb = qb == NQB - 1

            exp_cm = tc.tile_pool(name=f"exp{qb}", bufs=1)
            exp_pool = exp_cm.__enter__()
            expT = exp_pool.tile([P, KT, QB], f32r, name=f"exp{qb}")

            # -- prologue: qp^T for this q block --
            qp_cm = tc.tile_pool(name=f"qp{qb}", bufs=1)
            qp_pool = qp_cm.__enter__()
            qp = qp_pool.tile([P, ET, QB], f32r, name=f"qp{qb}")
            with tc.tile_pool(name=f"qpro{qb}", bufs=2) as ph, \
                 tc.tile_pool(name=f"qpro_ps{qb}", bufs=1, space="PSUM") as php:
                pss = [php.tile([P, QB], f32, tag=f"ps{m}", name=f"qpps{qb}_{m}")
                       for m in range(ET)]
                for t in range(DT):
                    wq_t = ph.tile([P, E], f32r, tag="wq_s", name=f"wq{qb}_{t}")
                    for h in range(2):
                        eng = nc.sync if h == 0 else nc.scalar
                        eng.dma_start(wq_t[:, h * E // 2:(h + 1) * E // 2],
                                      wq_r[:, t, h * E // 2:(h + 1) * E // 2])
                    qt_t = ph.tile([P, QB], f32r, tag="qT_s", name=f"qt{qb}_{t}")
                    nc.scalar.dma_start(qt_t[:], qT_r[:, t, q0:q0 + QB])
                    for m in range(ET):
                        mm(pss[m][:], wq_t[:, m * P:(m + 1) * P], qt_t[:],
                           t == 0, t == DT - 1)
                for m in range(ET):
                    nc.scalar.activation(qp[:, m, :], pss[m][:], AF.Identity,
                                         bias=bq_t[:, m:m + 1], scale=ISCALE)

            # -- logits + exp + softmax sum --
            with tc.tile_pool(name=f"lg_ps{qb}", bufs=4, space="PSUM") as php, \
                 tc.tile_pool(name=f"s_ps{qb}", bufs=1, space="PSUM") as sphp:
                s_ps = sphp.tile([P, QB], f32, name=f"sps{qb}")
                for kb in range(KT):
                    ps = php.tile([P, QB], f32, tag="ps", name=f"lgps{qb}_{kb}")
                    for e in range(ET):
                        mm(ps[:], kp[:, e, kb * P:(kb + 1) * P], qp[:, e, :],
                           e == 0, e == ET - 1)
                    nc.scalar.activation(expT[:, kb, :], ps[:], AF.Exp,
                                         bias=mask_t[:, kb:kb + 1])
                    mm(s_ps[:], ones_t[:], expT[:, kb, :], kb == 0, kb == KT - 1)
                nc.vector.reciprocal(recip_ts[qb][:], s_ps[:])

            qp_cm.__exit__(None, None, None)  # qp dead after logits

            # -- ctx accumulation --
            ctx_ps_cm = tc.tile_pool(name=f"ctx_ps{qb}", bufs=1, space="PSUM")
            ctx_php = ctx_ps_cm.__enter__()
            cps = [ctx_php.tile([P, QB], f32, tag=f"ps{e}", name=f"ctxps{qb}_{e}")
                   for e in range(ET)]
            for e in range(ET):
                for kb in range(KT):
                    mm(cps[e][:], vp[:, kb, e * P:(e + 1) * P], expT[:, kb, :],
                       kb == 0, kb == KT - 1)

            exp_cm.__exit__(None, None, None)  # expT consumed
            if last_qb:
                kp_cm.__exit__(None, None, None)  # kp dead after last logits

            # -- normalize into SBUF ctx^T --
            ctxs_cm = tc.tile_pool(name=f"ctxs{qb}", bufs=1)
            ctxs_pool = ctxs_cm.__enter__()
            ctx_sb = ctxs_pool.tile([P, ET, QB], f32r, name=f"ctx{qb}")
            for e in range(ET):
                nc.vector.tensor_mul(ctx_sb[:, e, :], cps[e][:], recip_ts[qb][:])
            ctx_ps_cm.__exit__(None, None, None)

            if not last_qb:
                dt_ = dram_pool.tile([P, ET, QB], f32r, name=f"ctxd{qb}")
                for e in range(ET):
                    nc.gpsimd.dma_start(dt_[:, e, :], ctx_sb[:, e, :])
                ctx_bounce.append(dt_)
                ctxs_cm.__exit__(None, None, None)
            else:
                ctx_bounce.append(None)
                ctx_last = ctx_sb
                ctx_last_cm = ctxs_cm

        # ---- out phase: out[q, :] = ctx @ ow + ob (ow streamed small) ----
        with tc.tile_pool(name="ctx_back", bufs=1) as cb, \
             tc.tile_pool(name="ow_s", bufs=8) as ows, \
             tc.tile_pool(name="outsb", bufs=6) as osb, \
             tc.tile_pool(name="out_ps", bufs=1, space="PSUM") as php:
            ob_t = cb.tile([P, D], f32, name="ob_t")
            nc.sync.dma_start(ob_t[:], ob_bc)
            ctx_ts = []
            for qb in range(NQB):
                if ctx_bounce[qb] is not None:
                    ctx_t = cb.tile([P, ET, QB], f32r, tag=f"cback{qb}",
                                    name=f"cb{qb}")
                    for e in range(ET):
                        nc.gpsimd.dma_start(ctx_t[:, e, :], ctx_bounce[qb][:, e, :])
                    ctx_ts.append(ctx_t)
                else:
                    ctx_ts.append(ctx_last)
            MQ = QB // P
            for nd in range(D // DNB):
                pss = {}
                for qb in range(NQB):
                    for mq in range(MQ):
                        pss[(qb, mq)] = php.tile(
                            [P, DNB], f32, tag=f"ps{qb}_{mq}",
                            name=f"ops{nd}_{qb}_{mq}")
                for e in range(ET):
                    ow_t = ows.tile([P, DNB], f32r, tag="ow_s",
                                    name=f"ow{nd}_{e}")
                    nc.sync.dma_start(ow_t[:],
                                      ow_r[:, e, nd * DNB:(nd + 1) * DNB])
                    for qb in range(NQB):
                        for mq in range(MQ):
                            mm(pss[(qb, mq)][:],
                               ctx_ts[qb][:, e, mq * P:(mq + 1) * P],
                               ow_t[:], e == 0, e == ET - 1)
                for qb in range(NQB):
                    for mq in range(MQ):
                        ot = osb.tile([P, DNB], f32, tag="ot",
                                      name=f"ot{nd}_{qb}_{mq}")
                        nc.vector.tensor_add(ot[:], pss[(qb, mq)][:],
                                             ob_t[:, nd * DNB:(nd + 1) * DNB])
                        nc.gpsimd.dma_start(
                            out[qb * QB + mq * P: qb * QB + (mq + 1) * P,
                                nd * DNB:(nd + 1) * DNB], ot[:])

        if ctx_last_cm is not None:
            ctx_last_cm.__exit__(None, None, None)
        vp_cm.__exit__(None, None, None)
        dram_cm.__exit__(None, None, None)
        smalls_cm.__exit__(None, None, None)

    nc.compile()
    return nc


def make_in_maps(v, k, q, mask, wq_w, wq_b, wk_w, wk_b, wv_w, wv_b, out_w, out_b,
                 n_cores=8, D=1024, E=1024, SK=2048, QSH=1024):
    """Host-side shard + layout prep (pure data movement, no math)."""
    ET = E // P
    KT = SK // P
    f = np.float32
    wq_w = np.ascontiguousarray(np.asarray(wq_w, f))
    wk_w = np.ascontiguousarray(np.asarray(wk_w, f))
    wv_w = np.ascontiguousarray(np.asarray(wv_w, f))
    out_w = np.ascontiguousarray(np.asarray(out_w, f))
    bq_col = np.ascontiguousarray(np.asarray(wq_b, f).reshape(ET, P).T)
    bk_col = np.ascontiguousarray(np.asarray(wk_b, f).reshape(ET, P).T)
    bv_bc = np.ascontiguousarray(np.broadcast_to(np.asarray(wv_b, f), (P, E)))
    ob_bc = np.ascontiguousarray(
        np.broadcast_to(np.asarray(out_b, f), (P, len(out_b))))
    ones_arr = np.ones((P, P), f)
    in_maps = []
    for c in range(n_cores):
        b, h = divmod(c, 2)
        qTc = np.ascontiguousarray(np.asarray(q[b, h * QSH:(h + 1) * QSH, :], f).T)
        kTc = np.ascontiguousarray(np.asarray(k[b], f).T)
        vTc = np.ascontiguousarray(np.asarray(v[b], f).T)
        mc = np.ascontiguousarray(np.asarray(mask[b, 0], f).reshape(KT, P).T)
        in_maps.append(dict(qT=qTc, kT=kTc, vT=vTc, mask_cols=mc,
                            ones_d=ones_arr,
                            wq=wq_w, wk=wk_w, wv=wv_w, ow=out_w,
                            bq_col=bq_col, bk_col=bk_col,
                            bv_bc=bv_bc, ob_bc=ob_bc))
    return in_maps


_NC_CACHE = {}


def kernel(v, k, q, mask, wq_w, wq_b, wk_w, wk_b, wv_w, wv_b, out_w, out_b):
    from concourse.bass_utils import run_bass_kernel_spmd

    B, S, D = 4, 2048, 1024
    E, QSH = 1024, 1024
    if "nc" not in _NC_CACHE:
        _NC_CACHE["nc"] = build_nc(D=D, E=E, SK=S, QSH=QSH, QB=512)
    nc = _NC_CACHE["nc"]

    in_maps = make_in_maps(v, k, q, mask, wq_w, wq_b, wk_w, wk_b, wv_w, wv_b,
                           out_w, out_b, n_cores=8, D=D, E=E, SK=S, QSH=QSH)
    trace = bool(int(os.environ.get("BASS_KERNEL_TRACE", "0")))
    res = run_bass_kernel_spmd(nc, in_maps, core_ids=list(range(8)), trace=trace)
    if trace:
        print(f"HW exec time: {res.exec_time_ns} ns")
        _NC_CACHE["last_exec_time_ns"] = res.exec_time_ns
        _NC_CACHE["last_trace"] = res.instructions_and_trace

    outp = np.empty((B, S, D), np.float32)
    for c in range(8):
        b, h = divmod(c, 2)
        outp[b, h * QSH:(h + 1) * QSH, :] = res.results[c]["out"]
    return outp



# revision 7
# speedup vs baseline: 1.2499x; 1.2499x over previous
"""Single-head attention (B=4, S=2048, D=E=1024) on 8 trn2 NeuronCores.

Sharding: data-parallel over (batch, q-half) -> 8 shards. Each core gets a
1024-row q shard plus the full 2048 keys of its batch; K/V projections are
recomputed on both cores of a batch pair (25% extra flops, zero collectives).

All matmul operands are bf16 (host-cast); PSUM accumulation stays fp32, so
per-value RMS error ~0.1% -- far inside the 2e-2 gate. bf16 runs at the same
1 cycle/row PE rate as fp32r but halves DMA + SBUF, which lets every weight
stay resident (no DRAM bounce) and keeps the PE streaming continuously:

  per-core PE work (cycles @2.4GHz):
    vp 131072 + kp 131072 + qp 65536 + logits 131072 + softmax-sum 16384
    + ctx 131072 + out 65536 = 672k cycles = 280.1us ideal

Schedule: vp -> kp -> qp(qb0) -> qb0 kb-loop [logits|exp|sum|ctx-half1, with
qp(qb1) in the spare PSUM bank] -> ctx-half2 -> qb1 kb-loop [with out(qb0) in
the spare bank] -> ctx-half2 -> out(qb1). PSUM never exceeds 8 banks; weights
for each phase are prefetched during the previous phase via sibling pools.
"""

import os
import numpy as np

P = 128
NEG = -1.0e9


def build_nc(D=1024, E=1024, SK=2048, QSH=1024, QB=512):
    """Build the per-core Bass module (SPMD; same program on all cores)."""
    import concourse.bass as bass
    import concourse.mybir as mybir
    import concourse.tile as tile
    from concourse import bacc

    f32 = mybir.dt.float32
    bf16 = mybir.dt.bfloat16
    AF = mybir.ActivationFunctionType

    DT = D // P          # contraction tiles over model dim
    ET = E // P          # enc tiles
    KT = SK // P         # key tiles
    NQB = QSH // QB      # q blocks (2)
    KNB = 512            # key free-dim block for kp
    DNB = 512            # model free-dim block for out
    MQ = QB // P         # q 128-row groups per block (4)
    ND = D // DNB        # out column chunks (2)
    ISCALE = 1.0 / float(np.sqrt(E))

    nc = bacc.Bacc(trn_type="TRN2")

    # ---- I/O (bf16 operands; f32 biases/mask; f32 output) ----
    qT = nc.dram_tensor("qT", [D, QSH], bf16, kind="ExternalInput")[:, :]
    kT = nc.dram_tensor("kT", [D, SK], bf16, kind="ExternalInput")[:, :]
    vT = nc.dram_tensor("vT", [D, SK], bf16, kind="ExternalInput")[:, :]
    mask_cols = nc.dram_tensor("mask_cols", [P, KT], f32, kind="ExternalInput")[:, :]
    ones_d = nc.dram_tensor("ones_d", [P, P], bf16, kind="ExternalInput")[:, :]
    wq = nc.dram_tensor("wq", [D, E], bf16, kind="ExternalInput")[:, :]
    wk = nc.dram_tensor("wk", [D, E], bf16, kind="ExternalInput")[:, :]
    wv = nc.dram_tensor("wv", [D, E], bf16, kind="ExternalInput")[:, :]
    ow = nc.dram_tensor("ow", [E, D], bf16, kind="ExternalInput")[:, :]
    bq_col = nc.dram_tensor("bq_col", [P, ET], f32, kind="ExternalInput")[:, :]
    bk_col = nc.dram_tensor("bk_col", [P, ET], f32, kind="ExternalInput")[:, :]
    bv_bc = nc.dram_tensor("bv_bc", [P, E], f32, kind="ExternalInput")[:, :]
    ob_bc = nc.dram_tensor("ob_bc", [P, D], f32, kind="ExternalInput")[:, :]
    out = nc.dram_tensor("out", [QSH, D], f32, kind="ExternalOutput")[:, :]

    qT_r = qT.rearrange("(t p) n -> p t n", p=P)   # [128, DT, QSH]
    kT_r = kT.rearrange("(t p) n -> p t n", p=P)
    vT_r = vT.rearrange("(t p) n -> p t n", p=P)
    wq_r = wq.rearrange("(t p) n -> p t n", p=P)   # [128, DT, E]
    wk_r = wk.rearrange("(t p) n -> p t n", p=P)
    wv_r = wv.rearrange("(t p) n -> p t n", p=P)
    ow_r = ow.rearrange("(t p) n -> p t n", p=P)   # [128, ET, D]

    def mm(ps, lhsT, rhs, start, stop):
        nc.tensor.matmul(ps, lhsT, rhs, start=start, stop=stop)

    with tile.TileContext(nc) as tc:
        # ---- persistent smalls ----
        with tc.tile_pool(name="smalls", bufs=1) as smalls:
            bv_t = smalls.tile([P, E], f32, name="bv_t")
            nc.scalar.dma_start(bv_t[:], bv_bc)
            mask_t = smalls.tile([P, KT], f32, name="maskc")
            nc.scalar.dma_start(mask_t[:], mask_cols)
            nc.scalar.mul(mask_t[:], mask_t[:], NEG)
            bk_t = smalls.tile([P, ET], f32, name="bkc")
            nc.scalar.dma_start(bk_t[:], bk_col)
            bq_t = smalls.tile([P, ET], f32, name="bqc")
            nc.scalar.dma_start(bq_t[:], bq_col)
            nc.scalar.mul(bq_t[:], bq_t[:], ISCALE)
            ones_t = smalls.tile([P, P], bf16, name="ones")
            nc.scalar.dma_start(ones_t[:], ones_d)
            recip_ts = [smalls.tile([P, QB], f32, name=f"recip{i}")
                        for i in range(NQB)]

            # persistent operand tensors
            with tc.tile_pool(name="wqp", bufs=1) as wqp, \
                 tc.tile_pool(name="vpp", bufs=1) as vpp, \
                 tc.tile_pool(name="kpp", bufs=1) as kpp, \
                 tc.tile_pool(name="qpp", bufs=1) as qpp, \
                 tc.tile_pool(name="expp", bufs=1) as expp, \
                 tc.tile_pool(name="ctxp", bufs=1) as ctxp:
                wq_t = wqp.tile([P, DT, E], bf16, name="wq_t")
                vp = vpp.tile([P, KT, E], bf16, name="vp")      # [k, E]
                kp = kpp.tile([P, ET, SK], bf16, name="kp")     # [E, k] (kp^T)
                qps = [qpp.tile([P, ET, QB], bf16, name=f"qp{i}")
                       for i in range(NQB)]                      # [E, q] (qp^T)
                expT = expp.tile([P, KT, QB], bf16, name="expT")  # [k, q]
                ctxs = [ctxp.tile([P, ET, QB], bf16, name=f"ctx{i}")
                        for i in range(NQB)]                     # [E, q] (ctx^T)

                # ============ phase A+B: vp then kp (sibling pools so kp
                # weights prefetch during vp) ============
                with tc.tile_pool(name="wv_w", bufs=1) as wvp, \
                     tc.tile_pool(name="wk_w", bufs=1) as wkp, \
                     tc.tile_pool(name="vT_s", bufs=3) as vts, \
                     tc.tile_pool(name="kT_s", bufs=2) as kts, \
                     tc.tile_pool(name="qT0_s", bufs=1) as qt0s, \
                     tc.tile_pool(name="ab_ps", bufs=3, space="PSUM") as abps:
                    wv_t = wvp.tile([P, DT, E], bf16, name="wv_t")
                    # wv by column halves: first psum groups (n=0) only need
                    # cols 0:512 of every t -> PE starts after 1MB, not 2MB
                    for nh in range(2):
                        for t in range(DT):
                            nc.sync.dma_start(
                                wv_t[:, t, nh * 512:(nh + 1) * 512],
                                wv_r[:, t, nh * 512:(nh + 1) * 512])
                    wk_t = wkp.tile([P, DT, E], bf16, name="wk_t")
                    for th in range(DT // 2):
                        nc.sync.dma_start(wk_t[:, 2 * th:2 * th + 2, :],
                                          wk_r[:, 2 * th:2 * th + 2, :])
                    for th in range(DT // 2):
                        nc.sync.dma_start(wq_t[:, 2 * th:2 * th + 2, :],
                                          wq_r[:, 2 * th:2 * th + 2, :])
                    qt0 = qt0s.tile([P, DT, QB], bf16, name="qt0")
                    for th in range(DT // 2):
                        nc.sync.dma_start(qt0[:, 2 * th:2 * th + 2, :],
                                          qT_r[:, 2 * th:2 * th + 2, 0:QB])

                    # -- vp: psum [128k, 512E] per (m, n) group --
                    vtiles = {}

                    def load_vt(mp):
                        vt = vts.tile([P, DT, 2 * P], bf16, tag="vt",
                                      name=f"vt{mp}")
                        if mp == 0:
                            nc.gpsimd.dma_start(vt[:, :, 0:P],
                                                vT_r[:, :, 0:P])
                            nc.gpsimd.dma_start(vt[:, :, P:2 * P],
                                                vT_r[:, :, P:2 * P])
                        else:
                            nc.gpsimd.dma_start(
                                vt[:], vT_r[:, :, 2 * mp * P:(2 * mp + 2) * P])
                        vtiles[mp] = vt

                    def vp_group(m, n):
                        ps = abps.tile([P, 512], f32, tag="ps",
                                       name=f"vps{m}_{n}")
                        vt = vtiles[m // 2]
                        mi = m % 2
                        for t in range(DT):
                            mm(ps[:], vt[:, t, mi * P:(mi + 1) * P],
                               wv_t[:, t, n * 512:(n + 1) * 512],
                               t == 0, t == DT - 1)
                        nc.vector.tensor_add(
                            vp[:, m, n * 512:(n + 1) * 512], ps[:],
                            bv_t[:, n * 512:(n + 1) * 512])

                    # pairs 0,1: n=0 groups first (col half 1 still loading)
                    load_vt(0)
                    load_vt(1)
                    for m in range(4):
                        vp_group(m, 0)
                    for m in range(4):
                        vp_group(m, 1)
                    for mp in range(2, KT // 2):
                        load_vt(mp)
                        for mi in range(2):
                            for n in range(E // 512):
                                vp_group(2 * mp + mi, n)

                    # -- kp: for each k-chunk, psum [128E, 512k] x8 --
                    for n in range(SK // KNB):
                        kt = kts.tile([P, DT, KNB], bf16, tag="kt",
                                      name=f"kt{n}")
                        nc.gpsimd.dma_start(kt[:, :, 0:KNB // 2],
                                            kT_r[:, :, n * KNB:n * KNB + KNB // 2])
                        nc.gpsimd.dma_start(kt[:, :, KNB // 2:KNB],
                                            kT_r[:, :, n * KNB + KNB // 2:(n + 1) * KNB])
                        for m in range(ET):
                            ps = abps.tile([P, KNB], f32, tag="ps",
                                           name=f"kps{n}_{m}")
                            for t in range(DT):
                                mm(ps[:], wk_t[:, t, m * P:(m + 1) * P],
                                   kt[:, t, :], t == 0, t == DT - 1)
                            nc.scalar.activation(
                                kp[:, m, n * KNB:(n + 1) * KNB], ps[:],
                                AF.Identity, bias=bk_t[:, m:m + 1])

                    # -- qp(qb0): psum [128E, 512q] x8 (reuse ab psum bufs) --
                    for m in range(ET):
                        ps = abps.tile([P, QB], f32, tag="ps", name=f"qps0_{m}")
                        for t in range(DT):
                            mm(ps[:], wq_t[:, t, m * P:(m + 1) * P],
                               qt0[:, t, :], t == 0, t == DT - 1)
                        nc.scalar.activation(qps[0][:, m, :], ps[:],
                                             AF.Identity,
                                             bias=bq_t[:, m:m + 1],
                                             scale=ISCALE)

                # ============ attention (ow/qT1/out-staging reuse AB space) ==
                with tc.tile_pool(name="ow_w", bufs=1) as owp, \
                     tc.tile_pool(name="qT1_s", bufs=1) as qt1s, \
                     tc.tile_pool(name="obp", bufs=1) as obp, \
                     tc.tile_pool(name="outsb", bufs=4) as osb:
                    ow_t = owp.tile([P, ET, D], bf16, name="ow_t")
                    for th in range(ET // 2):
                        nc.sync.dma_start(ow_t[:, 2 * th:2 * th + 2, :],
                                          ow_r[:, 2 * th:2 * th + 2, :])
                    ob_t = obp.tile([P, D], f32, name="ob_t")
                    nc.sync.dma_start(ob_t[:], ob_bc)
                    qt1 = qt1s.tile([P, DT, QB], bf16, name="qt1")
                    for th in range(DT // 2):
                        nc.sync.dma_start(qt1[:, 2 * th:2 * th + 2, :],
                                          qT_r[:, 2 * th:2 * th + 2, QB:2 * QB])

                    def out_group(qb, g, spare_pool):
                        """out[qb*QB+mq*128 : +128, nd*512 : +512] (8 mm)."""
                        nd, mq = divmod(g, MQ)
                        ps = spare_pool.tile([P, DNB], f32, tag="sp",
                                             name=f"ops{qb}_{g}")
                        for e in range(ET):
                            mm(ps[:], ctxs[qb][:, e, mq * P:(mq + 1) * P],
                               ow_t[:, e, nd * DNB:(nd + 1) * DNB],
                               e == 0, e == ET - 1)
                        ot = osb.tile([P, DNB], f32, tag="ot",
                                      name=f"ot{qb}_{g}")
                        nc.vector.tensor_add(
                            ot[:], ps[:], ob_t[:, nd * DNB:(nd + 1) * DNB])
                        nc.gpsimd.dma_start(
                            out[qb * QB + mq * P: qb * QB + (mq + 1) * P,
                                nd * DNB:(nd + 1) * DNB], ot[:])

                    for qb in range(NQB):
                        # banks 0-3: ctx half1 accumulators (live whole block)
                        cps_cm = tc.tile_pool(name=f"cps{qb}", bufs=1,
                                              space="PSUM")
                        cpsp = cps_cm.__enter__()
                        cps = [cpsp.tile([P, QB], f32, name=f"c{qb}_{e}")
                               for e in range(ET // 2)]
                        # banks 4-7: logits ping-pong, sum, spare
                        with tc.tile_pool(name=f"lg{qb}", bufs=2,
                                          space="PSUM") as lgp, \
                             tc.tile_pool(name=f"s{qb}", bufs=1,
                                          space="PSUM") as sp, \
                             tc.tile_pool(name=f"spare{qb}", bufs=1,
                                          space="PSUM") as spp:
                            s_ps = sp.tile([P, QB], f32, name=f"sps{qb}")

                            def lg_mm(kb):
                                ps = lgp.tile([P, QB], f32, tag="lg",
                                              name=f"lg{qb}_{kb}")
                                for e in range(ET):
                                    mm(ps[:], kp[:, e, kb * P:(kb + 1) * P],
                                       qps[qb][:, e, :], e == 0, e == ET - 1)
                                nc.scalar.activation(
                                    expT[:, kb, :], ps[:], AF.Exp,
                                    bias=mask_t[:, kb:kb + 1])

                            def tail_mm(kb):
                                mm(s_ps[:], ones_t[:], expT[:, kb, :],
                                   kb == 0, kb == KT - 1)
                                for e in range(ET // 2):
                                    mm(cps[e][:],
                                       vp[:, kb, e * P:(e + 1) * P],
                                       expT[:, kb, :], kb == 0, kb == KT - 1)

                            for kb in range(KT):
                                lg_mm(kb)
                                if kb > 0:
                                    tail_mm(kb - 1)
                                # spare-bank work, one group per slot:
                                if qb == 0 and kb >= ET:
                                    # qp(qb1) group m = kb-8
                                    m = kb - ET
                                    ps = spp.tile([P, QB], f32, tag="sp",
                                                  name=f"qps1_{m}")
                                    for t in range(DT):
                                        mm(ps[:], wq_t[:, t, m * P:(m + 1) * P],
                                           qt1[:, t, :], t == 0, t == DT - 1)
                                    nc.scalar.activation(
                                        qps[1][:, m, :], ps[:], AF.Identity,
                                        bias=bq_t[:, m:m + 1], scale=ISCALE)
                                if qb == 1 and kb % 2 == 1:
                                    out_group(0, kb // 2, spp)
                            tail_mm(KT - 1)
                            nc.vector.reciprocal(recip_ts[qb][:], s_ps[:])

                        # banks 4-7 now free -> ctx half2 accumulators
                        with tc.tile_pool(name=f"cps2_{qb}", bufs=1,
                                          space="PSUM") as cps2p:
                            cps2 = [cps2p.tile([P, QB], f32,
                                               name=f"c2_{qb}_{e}")
                                    for e in range(ET // 2)]
                            # evacuate half1 (DVE) while half2 accumulates
                            for e in range(ET // 2):
                                nc.vector.tensor_mul(ctxs[qb][:, e, :],
                                                     cps[e][:],
                                                     recip_ts[qb][:])
                            for ei in range(ET // 2):
                                e = ET // 2 + ei
                                for kb in range(KT):
                                    mm(cps2[ei][:],
                                       vp[:, kb, e * P:(e + 1) * P],
                                       expT[:, kb, :], kb == 0, kb == KT - 1)
                                # evac right away so the bank frees for the
                                # next qb's logits pools
                                nc.vector.tensor_mul(ctxs[qb][:, e, :],
                                                     cps2[ei][:],
                                                     recip_ts[qb][:])
                        cps_cm.__exit__(None, None, None)

                    # ---- final out phase: out(qb1) ----
                    with tc.tile_pool(name="out_ps", bufs=2,
                                      space="PSUM") as ops:
                        for g in range(ND * MQ):
                            out_group(1, g, ops)

    nc.compile()
    return nc


def make_in_maps(v, k, q, mask, wq_w, wq_b, wk_w, wk_b, wv_w, wv_b, out_w, out_b,
                 n_cores=8, D=1024, E=1024, SK=2048, QSH=1024):
    """Host-side shard + layout prep (data movement + bf16 cast, no math)."""
    import ml_dtypes
    bf = ml_dtypes.bfloat16
    ET = E // P
    KT = SK // P
    f = np.float32
    wq_w = np.ascontiguousarray(np.asarray(wq_w, f).astype(bf))
    wk_w = np.ascontiguousarray(np.asarray(wk_w, f).astype(bf))
    wv_w = np.ascontiguousarray(np.asarray(wv_w, f).astype(bf))
    out_w = np.ascontiguousarray(np.asarray(out_w, f).astype(bf))
    bq_col = np.ascontiguousarray(np.asarray(wq_b, f).reshape(ET, P).T)
    bk_col = np.ascontiguousarray(np.asarray(wk_b, f).reshape(ET, P).T)
    bv_bc = np.ascontiguousarray(np.broadcast_to(np.asarray(wv_b, f), (P, E)))
    ob_bc = np.ascontiguousarray(
        np.broadcast_to(np.asarray(out_b, f), (P, len(out_b))))
    ones_arr = np.ones((P, P), bf)
    in_maps = []
    for c in range(n_cores):
        b, h = divmod(c, 2)
        qTc = np.ascontiguousarray(
            np.asarray(q[b, h * QSH:(h + 1) * QSH, :], f).T.astype(bf))
        kTc = np.ascontiguousarray(np.asarray(k[b], f).T.astype(bf))
        vTc = np.ascontiguousarray(np.asarray(v[b], f).T.astype(bf))
        mc = np.ascontiguousarray(np.asarray(mask[b, 0], f).reshape(KT, P).T)
        in_maps.append(dict(qT=qTc, kT=kTc, vT=vTc, mask_cols=mc,
                            ones_d=ones_arr,
                            wq=wq_w, wk=wk_w, wv=wv_w, ow=out_w,
                            bq_col=bq_col, bk_col=bk_col,
                            bv_bc=bv_bc, ob_bc=ob_bc))
    return in_maps


_NC_CACHE = {}


def kernel(v, k, q, mask, wq_w, wq_b, wk_w, wk_b, wv_w, wv_b, out_w, out_b):
    from concourse.bass_utils import run_bass_kernel_spmd

    B, S, D = 4, 2048, 1024
    E, QSH = 1024, 1024
    if "nc" not in _NC_CACHE:
        _NC_CACHE["nc"] = build_nc(D=D, E=E, SK=S, QSH=QSH, QB=512)
    nc = _NC_CACHE["nc"]

    in_maps = make_in_maps(v, k, q, mask, wq_w, wq_b, wk_w, wk_b, wv_w, wv_b,
                           out_w, out_b, n_cores=8, D=D, E=E, SK=S, QSH=QSH)
    trace = bool(int(os.environ.get("BASS_KERNEL_TRACE", "0")))
    res = run_bass_kernel_spmd(nc, in_maps, core_ids=list(range(8)), trace=trace)
    if trace:
        print(f"HW exec time: {res.exec_time_ns} ns")
        _NC_CACHE["last_exec_time_ns"] = res.exec_time_ns
        _NC_CACHE["last_trace"] = res.instructions_and_trace

    outp = np.empty((B, S, D), np.float32)
    for c in range(8):
        b, h = divmod(c, 2)
        outp[b, h * QSH:(h + 1) * QSH, :] = res.results[c]["out"]
    return outp


# revision 47
# speedup vs baseline: 1.2857x; 1.0287x over previous
"""Single-head attention (B=4, S=2048, D=E=1024) on 8 trn2 NeuronCores.

Sharding: data-parallel over (batch, q-half) -> 8 shards. Each core gets a
1024-row q shard plus the full 2048 keys of its batch; K/V projections are
recomputed on both cores of a batch pair (25% extra flops, zero collectives).

All matmul operands are bf16 (host-cast); PSUM accumulation stays fp32, so
per-value RMS error ~0.1% -- far inside the 2e-2 gate. bf16 runs at the same
1 cycle/row PE rate as fp32r but halves DMA + SBUF, which lets every weight
stay resident (no DRAM bounce) and keeps the PE streaming continuously:

  per-core PE work (cycles @2.4GHz):
    vp 131072 + kp 131072 + qp 65536 + logits 131072 + softmax-sum 16384
    + ctx 131072 + out 65536 = 672k cycles = 280.1us ideal

Schedule: vp -> kp -> qp(qb0) -> qb0 kb-loop [logits|exp|sum|ctx-half1, with
qp(qb1) in the spare PSUM bank] -> ctx-half2 -> qb1 kb-loop [with out(qb0) in
the spare bank] -> ctx-half2 -> out(qb1). PSUM never exceeds 8 banks; weights
for each phase are prefetched during the previous phase via sibling pools.
"""

import os
import numpy as np

P = 128
NEG = -1.0e9


def build_nc(D=1024, E=1024, SK=2048, QSH=1024, QB=512):
    """Build the per-core Bass module (SPMD; same program on all cores)."""
    import concourse.bass as bass
    import concourse.mybir as mybir
    import concourse.tile as tile
    from concourse import bacc

    f32 = mybir.dt.float32
    bf16 = mybir.dt.bfloat16
    AF = mybir.ActivationFunctionType

    DT = D // P          # contraction tiles over model dim
    ET = E // P          # enc tiles
    KT = SK // P         # key tiles
    NQB = QSH // QB      # q blocks (2)
    KNB = 512            # key free-dim block for kp
    DNB = 512            # model free-dim block for out
    MQ = QB // P         # q 128-row groups per block (4)
    ND = D // DNB        # out column chunks (2)
    ISCALE = 1.0 / float(np.sqrt(E))

    nc = bacc.Bacc(trn_type="TRN2")

    # ---- I/O (bf16 operands; f32 biases/mask; f32 output) ----
    qT = nc.dram_tensor("qT", [D, QSH], bf16, kind="ExternalInput")[:, :]
    kT = nc.dram_tensor("kT", [D, SK], bf16, kind="ExternalInput")[:, :]
    vT = nc.dram_tensor("vT", [D, SK], bf16, kind="ExternalInput")[:, :]
    mask_cols = nc.dram_tensor("mask_cols", [P, KT], f32, kind="ExternalInput")[:, :]
    ones_d = nc.dram_tensor("ones_d", [P, P], bf16, kind="ExternalInput")[:, :]
    wq = nc.dram_tensor("wq", [D, E], bf16, kind="ExternalInput")[:, :]
    wk = nc.dram_tensor("wk", [D, E], bf16, kind="ExternalInput")[:, :]
    wv = nc.dram_tensor("wv", [D, E], bf16, kind="ExternalInput")[:, :]
    ow = nc.dram_tensor("ow", [E, D], bf16, kind="ExternalInput")[:, :]
    bq_col = nc.dram_tensor("bq_col", [P, ET], f32, kind="ExternalInput")[:, :]
    bk_col = nc.dram_tensor("bk_col", [P, ET], f32, kind="ExternalInput")[:, :]
    bv_bc = nc.dram_tensor("bv_bc", [P, E], bf16, kind="ExternalInput")[:, :]
    ob_bc = nc.dram_tensor("ob_bc", [P, D], f32, kind="ExternalInput")[:, :]
    # delta-row selector + ob with row 0 = out bias: the final out group
    # folds its bias in via matmul so the evac is a plain ACT copy
    ob_sel = nc.dram_tensor("ob_sel", [P, P], bf16, kind="ExternalInput")[:, :]
    ob_mat = nc.dram_tensor("ob_mat", [P, D], bf16, kind="ExternalInput")[:, :]
    out = nc.dram_tensor("out", [QSH, D], f32, kind="ExternalOutput")[:, :]

    qT_r = qT.rearrange("(t p) n -> p t n", p=P)   # [128, DT, QSH]
    kT_r = kT.rearrange("(t p) n -> p t n", p=P)
    vT_r = vT.rearrange("(t p) n -> p t n", p=P)
    wq_r = wq.rearrange("(t p) n -> p t n", p=P)   # [128, DT, E]
    wk_r = wk.rearrange("(t p) n -> p t n", p=P)
    wv_r = wv.rearrange("(t p) n -> p t n", p=P)
    ow_r = ow.rearrange("(t p) n -> p t n", p=P)   # [128, ET, D]

    def mm(ps, lhsT, rhs, start, stop):
        nc.tensor.matmul(ps, lhsT, rhs, start=start, stop=stop)

    NWARM = 10

    with tile.TileContext(nc) as tc:
        # ---- persistent smalls (tiles allocated here; DMAs emitted inside
        # the AB scope so the scalar queue prioritizes wv chunks) ----
        with tc.tile_pool(name="smalls", bufs=1) as smalls:
            bv_t = smalls.tile([P, E], bf16, name="bv_t")
            mask_t = smalls.tile([P, KT], f32, name="maskc")
            bk_t = smalls.tile([P, ET], f32, name="bkc")
            bq_t = smalls.tile([P, ET], f32, name="bqc")
            ones_t = smalls.tile([P, P], bf16, name="ones")
            recip_ts = [smalls.tile([P, QB], f32, name=f"recip{i}")
                        for i in range(NQB)]

            # persistent operand tensors
            with tc.tile_pool(name="wqp", bufs=1) as wqp, \
                 tc.tile_pool(name="vpp", bufs=1) as vpp, \
                 tc.tile_pool(name="kpp", bufs=1) as kpp, \
                 tc.tile_pool(name="qpp", bufs=1) as qpp, \
                 tc.tile_pool(name="expp", bufs=1) as expp, \
                 tc.tile_pool(name="ctxp", bufs=1) as ctxp:
                wq_t = wqp.tile([P, DT, E], bf16, name="wq_t")
                vp = vpp.tile([P, KT, E], bf16, name="vp")      # [k, E]
                kp = kpp.tile([P, ET, SK], bf16, name="kp")     # [E, k] (kp^T)
                qps = [qpp.tile([P, ET, QB], bf16, name=f"qp{i}")
                       for i in range(NQB)]                      # [E, q] (qp^T)
                expT = expp.tile([P, KT, QB], bf16, name="expT")  # [k, q]
                ctxs = [ctxp.tile([P, ET, QB], bf16, name=f"ctx{i}")
                        for i in range(NQB)]                     # [E, q] (ctx^T)

                # ============ phase A+B: vp then kp (sibling pools so kp
                # weights prefetch during vp) ============
                with tc.tile_pool(name="wv_w", bufs=1) as wvp, \
                     tc.tile_pool(name="wk_w", bufs=1) as wkp, \
                     tc.tile_pool(name="vT_s", bufs=2) as vts, \
                     tc.tile_pool(name="kT_s", bufs=2) as kts, \
                     tc.tile_pool(name="qT0_s", bufs=1) as qt0s, \
                     tc.tile_pool(name="warm", bufs=1) as warm, \
                     tc.tile_pool(name="warm_ps", bufs=1,
                                  space="PSUM") as wps, \
                     tc.tile_pool(name="ab_ps", bufs=3, space="PSUM") as abps:
                    # PE warm-up: dummy matmuls on a memset tile fill the
                    # ~5.7us wait for the first weight DMA and ramp the PE
                    # p-state so real work starts at full clock. Lives in
                    # the AB scope so nothing aliases (and WAR-waits on) it.
                    junk = warm.tile([P, 512], bf16, name="junk")
                    nc.vector.memset(junk[:], 0.0)
                    wp = wps.tile([P, 512], f32, name="warmps")
                    for i in range(NWARM):
                        nc.tensor.matmul(wp[:], junk[:, 0:P], junk[:],
                                         start=(i == 0),
                                         stop=(i == NWARM - 1))
                    # The shared DMA bus serves transfers in descriptor-gen
                    # completion order, so every queue is sequenced by first
                    # NEED: sync gets only the first wv half; Pool (slow 1.3us
                    # SWDGE gen each = natural pacing) carries the whole
                    # vp/kp-phase stream in consumption order; scalar gets the
                    # smalls then the late-needed wq/qt0.
                    wv_t = wvp.tile([P, DT, E], bf16, name="wv_t")
                    wk_t = wkp.tile([P, DT, E], bf16, name="wk_t")
                    nc.sync.dma_start(wv_t[:, 0:4, 0:512], wv_r[:, 0:4, 0:512])
                    nc.scalar.dma_start(wv_t[:, 4:8, 0:512],
                                        wv_r[:, 4:8, 0:512])
                    # smalls on scalar, ordered by first use (bv at ~8us)
                    nc.scalar.dma_start(bv_t[:, 0:512], bv_bc[:, 0:512])
                    nc.scalar.dma_start(bv_t[:, 512:1024], bv_bc[:, 512:1024])
                    nc.scalar.dma_start(mask_t[:], mask_cols)
                    nc.scalar.mul(mask_t[:], mask_t[:], NEG)
                    nc.scalar.dma_start(bk_t[:], bk_col)
                    nc.scalar.dma_start(bq_t[:], bq_col)
                    nc.scalar.mul(bq_t[:], bq_t[:], ISCALE)
                    nc.scalar.dma_start(ones_t[:], ones_d)
                    # wq/qt0 aren't needed until ~95us
                    for th in range(DT // 2):
                        nc.scalar.dma_start(wq_t[:, 2 * th:2 * th + 2, :],
                                            wq_r[:, 2 * th:2 * th + 2, :])
                    qt0 = qt0s.tile([P, DT, QB], bf16, name="qt0")
                    for th in range(DT // 2):
                        nc.scalar.dma_start(qt0[:, 2 * th:2 * th + 2, :],
                                            qT_r[:, 2 * th:2 * th + 2, 0:QB])

                    # -- vp: psum [128k, 512E] per (m, n) group; vT streamed
                    # in 1MB chunks of 4 k-tiles (desc count is per (p,t),
                    # so wider chunks halve Pool SWDGE time) --
                    vtiles = {}

                    def load_vt(c):
                        vt = vts.tile([P, DT, 4 * P], bf16, tag="vt",
                                      name=f"vt{c}")
                        if c == 0:
                            # split so the very first k-tile lands early
                            nc.gpsimd.dma_start(vt[:, :, 0:P],
                                                vT_r[:, :, 0:P])
                            nc.gpsimd.dma_start(vt[:, :, P:4 * P],
                                                vT_r[:, :, P:4 * P])
                        else:
                            nc.gpsimd.dma_start(
                                vt[:], vT_r[:, :, 4 * c * P:(4 * c + 4) * P])
                        vtiles[c] = vt

                    def vp_group(m, n):
                        ps = abps.tile([P, 512], f32, tag="ps",
                                       name=f"vps{m}_{n}")
                        vt = vtiles[m // 4]
                        mi = m % 4
                        for t in range(DT):
                            mm(ps[:], vt[:, t, mi * P:(mi + 1) * P],
                               wv_t[:, t, n * 512:(n + 1) * 512],
                               t == 0, t == DT - 1)
                        nc.vector.tensor_add(
                            vp[:, m, n * 512:(n + 1) * 512], ps[:],
                            bv_t[:, n * 512:(n + 1) * 512])

                    # chunk 0: n=0 groups first (wv col half 1 still loading)
                    load_vt(0)
                    # Pool queue, in bus-need order: vt0, wv col1, vt1, wk
                    nc.gpsimd.dma_start(wv_t[:, :, 512:1024],
                                        wv_r[:, :, 512:1024])
                    load_vt(1)
                    nc.gpsimd.dma_start(wk_t[:, :, 0:512], wk_r[:, :, 0:512])
                    nc.gpsimd.dma_start(wk_t[:, :, 512:1024],
                                        wk_r[:, :, 512:1024])
                    for m in range(4):
                        vp_group(m, 0)
                    for m in range(4):
                        vp_group(m, 1)
                    for c in range(1, KT // 4):
                        if c + 1 < KT // 4:
                            load_vt(c + 1)
                        for mi in range(4):
                            for n in range(E // 512):
                                vp_group(4 * c + mi, n)

                    # -- kp: for each k-chunk, psum [128E, 512k] x8 --
                    for n in range(SK // KNB):
                        kt = kts.tile([P, DT, KNB], bf16, tag="kt",
                                      name=f"kt{n}")
                        nc.gpsimd.dma_start(kt[:],
                                            kT_r[:, :, n * KNB:(n + 1) * KNB])
                        for m in range(ET):
                            ps = abps.tile([P, KNB], f32, tag="ps",
                                           name=f"kps{n}_{m}")
                            for t in range(DT):
                                mm(ps[:], wk_t[:, t, m * P:(m + 1) * P],
                                   kt[:, t, :], t == 0, t == DT - 1)
                            nc.scalar.activation(
                                kp[:, m, n * KNB:(n + 1) * KNB], ps[:],
                                AF.Identity, bias=bk_t[:, m:m + 1])

                    # -- qp(qb0): psum [128E, 512q] x8 (reuse ab psum bufs) --
                    for m in range(ET):
                        ps = abps.tile([P, QB], f32, tag="ps", name=f"qps0_{m}")
                        for t in range(DT):
                            mm(ps[:], wq_t[:, t, m * P:(m + 1) * P],
                               qt0[:, t, :], t == 0, t == DT - 1)
                        nc.scalar.activation(qps[0][:, m, :], ps[:],
                                             AF.Identity,
                                             bias=bq_t[:, m:m + 1],
                                             scale=ISCALE)

                # ============ attention (ow/qT1/out-staging reuse AB space) ==
                with tc.tile_pool(name="ow_w", bufs=1) as owp, \
                     tc.tile_pool(name="qT1_s", bufs=1) as qt1s, \
                     tc.tile_pool(name="obp", bufs=1) as obp, \
                     tc.tile_pool(name="outsb", bufs=4) as osb:
                    # scalar queue (slow SWDGE gen) paces these behind
                    # wq/qt0 so they don't jump the bus ahead of the vp/kp
                    # streams (none are needed before ~150us)
                    qt1 = qt1s.tile([P, DT, QB], bf16, name="qt1")
                    for th in range(DT // 2):
                        nc.scalar.dma_start(qt1[:, 2 * th:2 * th + 2, :],
                                            qT_r[:, 2 * th:2 * th + 2,
                                                 QB:2 * QB])
                    ow_t = owp.tile([P, ET, D], bf16, name="ow_t")
                    for th in range(ET // 2):
                        nc.scalar.dma_start(ow_t[:, 2 * th:2 * th + 2, :],
                                            ow_r[:, 2 * th:2 * th + 2, :])
                    ob_t = obp.tile([P, D], f32, name="ob_t")
                    nc.scalar.dma_start(ob_t[:], ob_bc)
                    obsel_t = obp.tile([P, P], bf16, name="obsel_t")
                    nc.scalar.dma_start(obsel_t[:], ob_sel)
                    obmat_t = obp.tile([P, D], bf16, name="obmat_t")
                    nc.scalar.dma_start(obmat_t[:], ob_mat)

                    def out_group(qb, g, spare_pool, split=False):
                        """out[qb*QB+mq*128 : +128, nd*512 : +512] (8 mm)."""
                        nd, mq = divmod(g, MQ)
                        ps = spare_pool.tile([P, DNB], f32, tag="sp",
                                             name=f"ops{qb}_{g}")
                        for e in range(ET):
                            mm(ps[:], ctxs[qb][:, e, mq * P:(mq + 1) * P],
                               ow_t[:, e, nd * DNB:(nd + 1) * DNB],
                               e == 0, e == ET - 1)
                        r0 = qb * QB + mq * P
                        ot = osb.tile([P, DNB], f32, tag="ot",
                                      name=f"ot{qb}_{g}")
                        nc.vector.tensor_add(
                            ot[:], ps[:], ob_t[:, nd * DNB:(nd + 1) * DNB])
                        nc.gpsimd.dma_start(
                            out[r0:r0 + P, nd * DNB:(nd + 1) * DNB], ot[:])

                    def out_group_tail(qb, g, tail_pool):
                        """Last group: bias folded in via the delta-row
                        matmul, evac by plain ACT copy (no DVE bias add on
                        the critical path), 2 column chunks so chunk 0's
                        store overlaps chunk 1's matmuls; final store rides
                        the fast HWDGE queue."""
                        nd, mq = divmod(g, MQ)
                        r0 = qb * QB + mq * P
                        engs = [nc.gpsimd, nc.sync]
                        for j in range(2):
                            c0 = nd * DNB + j * (DNB // 2)
                            w = DNB // 2
                            ps = tail_pool.tile([P, w], f32, tag=f"tp{j}",
                                                name=f"opst{qb}_{g}_{j}")
                            ot = osb.tile([P, w], f32, tag=f"ott{j}",
                                          name=f"ott{qb}_{g}_{j}")
                            mm(ps[:], obsel_t[:], obmat_t[:, c0:c0 + w],
                               True, False)
                            for e in range(ET):
                                mm(ps[:],
                                   ctxs[qb][:, e, mq * P:(mq + 1) * P],
                                   ow_t[:, e, c0:c0 + w],
                                   False, e == ET - 1)
                            nc.scalar.activation(ot[:], ps[:], AF.Identity)
                            engs[j].dma_start(
                                out[r0:r0 + P, c0:c0 + w], ot[:])

                    for qb in range(NQB):
                        # banks 0-3: ctx half1 accumulators (live whole block)
                        cps_cm = tc.tile_pool(name=f"cps{qb}", bufs=1,
                                              space="PSUM")
                        cpsp = cps_cm.__enter__()
                        cps = [cpsp.tile([P, QB], f32, name=f"c{qb}_{e}")
                               for e in range(ET // 2)]
                        # banks 4-7: logits ping-pong, sum, spare
                        with tc.tile_pool(name=f"lg{qb}", bufs=2,
                                          space="PSUM") as lgp, \
                             tc.tile_pool(name=f"s{qb}", bufs=1,
                                          space="PSUM") as sp, \
                             tc.tile_pool(name=f"spare{qb}", bufs=1,
                                          space="PSUM") as spp:
                            s_ps = sp.tile([P, QB], f32, name=f"sps{qb}")

                            def lg_mm(kb):
                                ps = lgp.tile([P, QB], f32, tag="lg",
                                              name=f"lg{qb}_{kb}")
                                for e in range(ET):
                                    mm(ps[:], kp[:, e, kb * P:(kb + 1) * P],
                                       qps[qb][:, e, :], e == 0, e == ET - 1)
                                nc.scalar.activation(
                                    expT[:, kb, :], ps[:], AF.Exp,
                                    bias=mask_t[:, kb:kb + 1])

                            def tail_mm(kb):
                                mm(s_ps[:], ones_t[:], expT[:, kb, :],
                                   kb == 0, kb == KT - 1)
                                for e in range(ET // 2):
                                    mm(cps[e][:],
                                       vp[:, kb, e * P:(e + 1) * P],
                                       expT[:, kb, :], kb == 0, kb == KT - 1)

                            for kb in range(KT):
                                lg_mm(kb)
                                if kb > 0:
                                    tail_mm(kb - 1)
                                # spare-bank work, one group per slot:
                                if qb == 0 and kb >= ET:
                                    # qp(qb1) group m = kb-8
                                    m = kb - ET
                                    ps = spp.tile([P, QB], f32, tag="sp",
                                                  name=f"qps1_{m}")
                                    for t in range(DT):
                                        mm(ps[:], wq_t[:, t, m * P:(m + 1) * P],
                                           qt1[:, t, :], t == 0, t == DT - 1)
                                    nc.scalar.activation(
                                        qps[1][:, m, :], ps[:], AF.Identity,
                                        bias=bq_t[:, m:m + 1], scale=ISCALE)
                                if qb == 1 and kb % 2 == 1:
                                    out_group(0, kb // 2, spp)
                            tail_mm(KT - 1)
                            nc.vector.reciprocal(recip_ts[qb][:], s_ps[:])

                        # banks 4-7 now free -> ctx half2 accumulators
                        with tc.tile_pool(name=f"cps2_{qb}", bufs=1,
                                          space="PSUM") as cps2p:
                            cps2 = [cps2p.tile([P, QB], f32,
                                               name=f"c2_{qb}_{e}")
                                    for e in range(ET // 2)]
                            # evacuate half1 (DVE) while half2 accumulates
                            for e in range(ET // 2):
                                nc.vector.tensor_mul(ctxs[qb][:, e, :],
                                                     cps[e][:],
                                                     recip_ts[qb][:])
                            for ei in range(ET // 2):
                                e = ET // 2 + ei
                                for kb in range(KT):
                                    mm(cps2[ei][:],
                                       vp[:, kb, e * P:(e + 1) * P],
                                       expT[:, kb, :], kb == 0, kb == KT - 1)
                                # evac right away so the bank frees for the
                                # next qb's logits pools
                                nc.vector.tensor_mul(ctxs[qb][:, e, :],
                                                     cps2[ei][:],
                                                     recip_ts[qb][:])
                        cps_cm.__exit__(None, None, None)

                    # ---- final out phase: out(qb1) ----
                    with tc.tile_pool(name="out_ps", bufs=2,
                                      space="PSUM") as ops, \
                         tc.tile_pool(name="tail_ps", bufs=1,
                                      space="PSUM") as tps:
                        for g in range(ND * MQ - 1):
                            out_group(1, g, ops)
                        out_group_tail(1, ND * MQ - 1, tps)

    nc.compile()
    return nc


def make_in_maps(v, k, q, mask, wq_w, wq_b, wk_w, wk_b, wv_w, wv_b, out_w, out_b,
                 n_cores=8, D=1024, E=1024, SK=2048, QSH=1024):
    """Host-side shard + layout prep (data movement + bf16 cast, no math)."""
    import ml_dtypes
    bf = ml_dtypes.bfloat16
    ET = E // P
    KT = SK // P
    f = np.float32
    wq_w = np.ascontiguousarray(np.asarray(wq_w, f).astype(bf))
    wk_w = np.ascontiguousarray(np.asarray(wk_w, f).astype(bf))
    wv_w = np.ascontiguousarray(np.asarray(wv_w, f).astype(bf))
    out_w = np.ascontiguousarray(np.asarray(out_w, f).astype(bf))
    bq_col = np.ascontiguousarray(np.asarray(wq_b, f).reshape(ET, P).T)
    bk_col = np.ascontiguousarray(np.asarray(wk_b, f).reshape(ET, P).T)
    bv_bc = np.ascontiguousarray(
        np.broadcast_to(np.asarray(wv_b, f).astype(bf), (P, E)))
    ob_bc = np.ascontiguousarray(
        np.broadcast_to(np.asarray(out_b, f), (P, len(out_b))))
    ones_arr = np.ones((P, P), bf)
    ob_sel = np.zeros((P, P), bf)
    ob_sel[0, :] = 1
    ob_mat = np.zeros((P, len(out_b)), bf)
    ob_mat[0, :] = np.asarray(out_b, f).astype(bf)
    in_maps = []
    for c in range(n_cores):
        b, h = divmod(c, 2)
        qTc = np.ascontiguousarray(
            np.asarray(q[b, h * QSH:(h + 1) * QSH, :], f).T.astype(bf))
        kTc = np.ascontiguousarray(np.asarray(k[b], f).T.astype(bf))
        vTc = np.ascontiguousarray(np.asarray(v[b], f).T.astype(bf))
        mc = np.ascontiguousarray(np.asarray(mask[b, 0], f).reshape(KT, P).T)
        in_maps.append(dict(qT=qTc, kT=kTc, vT=vTc, mask_cols=mc,
                            ones_d=ones_arr,
                            wq=wq_w, wk=wk_w, wv=wv_w, ow=out_w,
                            bq_col=bq_col, bk_col=bk_col,
                            bv_bc=bv_bc, ob_bc=ob_bc,
                            ob_sel=ob_sel, ob_mat=ob_mat))
    return in_maps


_NC_CACHE = {}


def kernel(v, k, q, mask, wq_w, wq_b, wk_w, wk_b, wv_w, wv_b, out_w, out_b):
    from concourse.bass_utils import run_bass_kernel_spmd

    B, S, D = 4, 2048, 1024
    E, QSH = 1024, 1024
    if "nc" not in _NC_CACHE:
        _NC_CACHE["nc"] = build_nc(D=D, E=E, SK=S, QSH=QSH, QB=512)
    nc = _NC_CACHE["nc"]

    in_maps = make_in_maps(v, k, q, mask, wq_w, wq_b, wk_w, wk_b, wv_w, wv_b,
                           out_w, out_b, n_cores=8, D=D, E=E, SK=S, QSH=QSH)
    trace = bool(int(os.environ.get("BASS_KERNEL_TRACE", "0")))
    res = run_bass_kernel_spmd(nc, in_maps, core_ids=list(range(8)), trace=trace)
    if trace:
        print(f"HW exec time: {res.exec_time_ns} ns")
        _NC_CACHE["last_exec_time_ns"] = res.exec_time_ns
        _NC_CACHE["last_trace"] = res.instructions_and_trace

    outp = np.empty((B, S, D), np.float32)
    for c in range(8):
        b, h = divmod(c, 2)
        outp[b, h * QSH:(h + 1) * QSH, :] = res.results[c]["out"]
    return outp


# revision 57
# speedup vs baseline: 1.2998x; 1.0110x over previous
"""Single-head attention (B=4, S=2048, D=E=1024) on 8 trn2 NeuronCores.

Sharding: data-parallel over (batch, q-half) -> 8 shards. Each core gets a
1024-row q shard plus the full 2048 keys of its batch; K/V projections are
recomputed on both cores of a batch pair (25% extra flops, zero collectives).

All matmul operands are bf16 (host-cast); PSUM accumulation stays fp32, so
per-value RMS error ~0.1% -- far inside the 2e-2 gate. bf16 runs at the same
1 cycle/row PE rate as fp32r but halves DMA + SBUF, which lets every weight
stay resident (no DRAM bounce) and keeps the PE streaming continuously:

  per-core PE work (cycles @2.4GHz):
    vp 131072 + kp 131072 + qp 65536 + logits 131072 + softmax-sum 16384
    + ctx 131072 + out 65536 = 672k cycles = 280.1us ideal

Schedule: vp -> kp -> qp(qb0) -> qb0 kb-loop [logits|exp|sum|ctx-half1, with
qp(qb1) in the spare PSUM bank] -> ctx-half2 -> qb1 kb-loop [with out(qb0) in
the spare bank] -> ctx-half2 -> out(qb1). PSUM never exceeds 8 banks; weights
for each phase are prefetched during the previous phase via sibling pools.
"""

import os
import numpy as np

P = 128
NEG = -1.0e9


def build_nc(D=1024, E=1024, SK=2048, QSH=1024, QB=512):
    """Build the per-core Bass module (SPMD; same program on all cores)."""
    import concourse.bass as bass
    import concourse.mybir as mybir
    import concourse.tile as tile
    from concourse import bacc

    f32 = mybir.dt.float32
    bf16 = mybir.dt.bfloat16
    AF = mybir.ActivationFunctionType

    DT = D // P          # contraction tiles over model dim
    ET = E // P          # enc tiles
    KT = SK // P         # key tiles
    NQB = QSH // QB      # q blocks (2)
    KNB = 512            # key free-dim block for kp
    DNB = 512            # model free-dim block for out
    MQ = QB // P         # q 128-row groups per block (4)
    ND = D // DNB        # out column chunks (2)
    ISCALE = 1.0 / float(np.sqrt(E))

    nc = bacc.Bacc(trn_type="TRN2")

    # ---- I/O (bf16 operands; f32 biases/mask; f32 output) ----
    qT = nc.dram_tensor("qT", [D, QSH], bf16, kind="ExternalInput")[:, :]
    kT = nc.dram_tensor("kT", [D, SK], bf16, kind="ExternalInput")[:, :]
    vT = nc.dram_tensor("vT", [D, SK], bf16, kind="ExternalInput")[:, :]
    mask_cols = nc.dram_tensor("mask_cols", [P, KT], f32, kind="ExternalInput")[:, :]
    ones_d = nc.dram_tensor("ones_d", [P, P], bf16, kind="ExternalInput")[:, :]
    wq = nc.dram_tensor("wq", [D, E], bf16, kind="ExternalInput")[:, :]
    wk = nc.dram_tensor("wk", [D, E], bf16, kind="ExternalInput")[:, :]
    wv = nc.dram_tensor("wv", [D, E], bf16, kind="ExternalInput")[:, :]
    ow = nc.dram_tensor("ow", [E, D], bf16, kind="ExternalInput")[:, :]
    bq_col = nc.dram_tensor("bq_col", [P, ET], f32, kind="ExternalInput")[:, :]
    bk_col = nc.dram_tensor("bk_col", [P, ET], f32, kind="ExternalInput")[:, :]
    bv_bc = nc.dram_tensor("bv_bc", [P, E], bf16, kind="ExternalInput")[:, :]
    ob_bc = nc.dram_tensor("ob_bc", [P, D], f32, kind="ExternalInput")[:, :]
    # delta-row selector + ob with row 0 = out bias: the final out group
    # folds its bias in via matmul so the evac is a plain ACT copy
    ob_sel = nc.dram_tensor("ob_sel", [P, P], bf16, kind="ExternalInput")[:, :]
    ob_mat = nc.dram_tensor("ob_mat", [P, D], bf16, kind="ExternalInput")[:, :]
    out = nc.dram_tensor("out", [QSH, D], f32, kind="ExternalOutput")[:, :]

    qT_r = qT.rearrange("(t p) n -> p t n", p=P)   # [128, DT, QSH]
    kT_r = kT.rearrange("(t p) n -> p t n", p=P)
    vT_r = vT.rearrange("(t p) n -> p t n", p=P)
    wq_r = wq.rearrange("(t p) n -> p t n", p=P)   # [128, DT, E]
    wk_r = wk.rearrange("(t p) n -> p t n", p=P)
    wv_r = wv.rearrange("(t p) n -> p t n", p=P)
    ow_r = ow.rearrange("(t p) n -> p t n", p=P)   # [128, ET, D]

    def mm(ps, lhsT, rhs, start, stop):
        nc.tensor.matmul(ps, lhsT, rhs, start=start, stop=stop)

    NWARM = 10

    with tile.TileContext(nc) as tc:
        # ---- persistent smalls (tiles allocated here; DMAs emitted inside
        # the AB scope so the scalar queue prioritizes wv chunks) ----
        with tc.tile_pool(name="smalls", bufs=1) as smalls:
            bv_t = smalls.tile([P, E], bf16, name="bv_t")
            mask_t = smalls.tile([P, KT], f32, name="maskc")
            bk_t = smalls.tile([P, ET], f32, name="bkc")
            bq_t = smalls.tile([P, ET], f32, name="bqc")
            ones_t = smalls.tile([P, P], bf16, name="ones")
            recip_ts = [smalls.tile([P, QB], f32, name=f"recip{i}")
                        for i in range(NQB)]

            # persistent operand tensors
            with tc.tile_pool(name="wqp", bufs=1) as wqp, \
                 tc.tile_pool(name="vpp", bufs=1) as vpp, \
                 tc.tile_pool(name="kpp", bufs=1) as kpp, \
                 tc.tile_pool(name="qpp", bufs=1) as qpp, \
                 tc.tile_pool(name="expp", bufs=1) as expp, \
                 tc.tile_pool(name="ctxp", bufs=1) as ctxp:
                wq_t = wqp.tile([P, DT, E], bf16, name="wq_t")
                vp = vpp.tile([P, KT, E], bf16, name="vp")      # [k, E]
                kp = kpp.tile([P, ET, SK], bf16, name="kp")     # [E, k] (kp^T)
                qps = [qpp.tile([P, ET, QB], bf16, name=f"qp{i}")
                       for i in range(NQB)]                      # [E, q] (qp^T)
                expT = expp.tile([P, KT, QB], bf16, name="expT")  # [k, q]
                ctxs = [ctxp.tile([P, ET, QB], bf16, name=f"ctx{i}")
                        for i in range(NQB)]                     # [E, q] (ctx^T)

                # ============ phase A+B: vp then kp (sibling pools so kp
                # weights prefetch during vp) ============
                with tc.tile_pool(name="wv_w", bufs=1) as wvp, \
                     tc.tile_pool(name="wk_w", bufs=1) as wkp, \
                     tc.tile_pool(name="vT_s", bufs=2) as vts, \
                     tc.tile_pool(name="kT_s", bufs=2) as kts, \
                     tc.tile_pool(name="qT0_s", bufs=1) as qt0s, \
                     tc.tile_pool(name="warm", bufs=1) as warm, \
                     tc.tile_pool(name="warm_ps", bufs=1,
                                  space="PSUM") as wps, \
                     tc.tile_pool(name="ab_ps", bufs=3, space="PSUM") as abps:
                    # PE warm-up: dummy matmuls on a memset tile fill the
                    # ~5.7us wait for the first weight DMA and ramp the PE
                    # p-state so real work starts at full clock. Lives in
                    # the AB scope so nothing aliases (and WAR-waits on) it.
                    junk = warm.tile([P, 512], bf16, name="junk")
                    nc.vector.memset(junk[:], 0.0)
                    wp = wps.tile([P, 512], f32, name="warmps")
                    for i in range(NWARM):
                        nc.tensor.matmul(wp[:], junk[:, 0:P], junk[:],
                                         start=(i == 0),
                                         stop=(i == NWARM - 1))
                    # The shared DMA bus serves transfers in descriptor-gen
                    # completion order, so every queue is sequenced by first
                    # NEED: sync gets only the first wv half; Pool (slow 1.3us
                    # SWDGE gen each = natural pacing) carries the whole
                    # vp/kp-phase stream in consumption order; scalar gets the
                    # smalls then the late-needed wq/qt0.
                    wv_t = wvp.tile([P, DT, E], bf16, name="wv_t")
                    wk_t = wkp.tile([P, DT, E], bf16, name="wk_t")
                    nc.sync.dma_start(wv_t[:, 0:4, 0:512], wv_r[:, 0:4, 0:512])
                    nc.scalar.dma_start(wv_t[:, 4:8, 0:512],
                                        wv_r[:, 4:8, 0:512])
                    # smalls on scalar, ordered by first use (bv col0 at
                    # ~13us; bv col1 rides the Pool queue after wv col1)
                    nc.scalar.dma_start(bv_t[:, 0:512], bv_bc[:, 0:512])
                    nc.scalar.dma_start(mask_t[:], mask_cols)
                    nc.scalar.mul(mask_t[:], mask_t[:], NEG)
                    nc.scalar.dma_start(bk_t[:], bk_col)
                    nc.scalar.dma_start(bq_t[:], bq_col)
                    nc.scalar.mul(bq_t[:], bq_t[:], ISCALE)
                    nc.scalar.dma_start(ones_t[:], ones_d)
                    qt0 = qt0s.tile([P, DT, QB], bf16, name="qt0")

                    # -- vp: psum [128k, 512E] per (m, n) group; vT streamed
                    # in 1MB chunks of 4 k-tiles (desc count is per (p,t),
                    # so wider chunks halve Pool SWDGE time) --
                    vtiles = {}

                    def load_vt(c):
                        vt = vts.tile([P, DT, 4 * P], bf16, tag="vt",
                                      name=f"vt{c}")
                        if c == 0:
                            # split so each early k-tile unblocks as it lands
                            nc.gpsimd.dma_start(vt[:, :, 0:P],
                                                vT_r[:, :, 0:P])
                            nc.gpsimd.dma_start(vt[:, :, P:2 * P],
                                                vT_r[:, :, P:2 * P])
                            nc.gpsimd.dma_start(vt[:, :, 2 * P:4 * P],
                                                vT_r[:, :, 2 * P:4 * P])
                        else:
                            nc.gpsimd.dma_start(
                                vt[:], vT_r[:, :, 4 * c * P:(4 * c + 4) * P])
                        vtiles[c] = vt

                    def vp_group(m, n):
                        ps = abps.tile([P, 512], f32, tag="ps",
                                       name=f"vps{m}_{n}")
                        vt = vtiles[m // 4]
                        mi = m % 4
                        for t in range(DT):
                            mm(ps[:], vt[:, t, mi * P:(mi + 1) * P],
                               wv_t[:, t, n * 512:(n + 1) * 512],
                               t == 0, t == DT - 1)
                        nc.vector.tensor_add(
                            vp[:, m, n * 512:(n + 1) * 512], ps[:],
                            bv_t[:, n * 512:(n + 1) * 512])

                    # chunk 0: n=0 groups first (wv col half 1 still loading)
                    load_vt(0)
                    # Pool/SWDGE queue carries everything else in strict
                    # consumption order (scalar/sync are HWDGE-fast and would
                    # let late-needed weights jump the shared bus)
                    nc.gpsimd.dma_start(wv_t[:, :, 512:1024],
                                        wv_r[:, :, 512:1024])
                    nc.gpsimd.dma_start(bv_t[:, 512:1024], bv_bc[:, 512:1024])
                    load_vt(1)
                    for m in range(4):
                        vp_group(m, 0)
                    for m in range(4):
                        vp_group(m, 1)
                    for c in range(1, KT // 4):
                        if c + 1 < KT // 4:
                            load_vt(c + 1)
                        for mi in range(4):
                            for n in range(E // 512):
                                vp_group(4 * c + mi, n)
                    # wk lands on the bus after vt2/vt3 (needed at kp start)
                    nc.gpsimd.dma_start(wk_t[:, :, 0:512], wk_r[:, :, 0:512])
                    nc.gpsimd.dma_start(wk_t[:, :, 512:1024],
                                        wk_r[:, :, 512:1024])

                    # -- kp: for each k-chunk, psum [128E, 512k] x8 --
                    for n in range(SK // KNB):
                        kt = kts.tile([P, DT, KNB], bf16, tag="kt",
                                      name=f"kt{n}")
                        nc.gpsimd.dma_start(kt[:],
                                            kT_r[:, :, n * KNB:(n + 1) * KNB])
                        for m in range(ET):
                            ps = abps.tile([P, KNB], f32, tag="ps",
                                           name=f"kps{n}_{m}")
                            for t in range(DT):
                                mm(ps[:], wk_t[:, t, m * P:(m + 1) * P],
                                   kt[:, t, :], t == 0, t == DT - 1)
                            nc.scalar.activation(
                                kp[:, m, n * KNB:(n + 1) * KNB], ps[:],
                                AF.Identity, bias=bk_t[:, m:m + 1])

                    # wq/qt0 queue behind the kp stream; kt2/kt3's WAR head-
                    # of-line block paces their generation to ~90us, arriving
                    # in time for qp0 at ~116us
                    nc.gpsimd.dma_start(wq_t[:, 0:4, :], wq_r[:, 0:4, :])
                    nc.gpsimd.dma_start(wq_t[:, 4:8, :], wq_r[:, 4:8, :])
                    nc.gpsimd.dma_start(qt0[:], qT_r[:, :, 0:QB])

                    # -- qp(qb0): psum [128E, 512q] x8 (reuse ab psum bufs) --
                    for m in range(ET):
                        ps = abps.tile([P, QB], f32, tag="ps", name=f"qps0_{m}")
                        for t in range(DT):
                            mm(ps[:], wq_t[:, t, m * P:(m + 1) * P],
                               qt0[:, t, :], t == 0, t == DT - 1)
                        nc.scalar.activation(qps[0][:, m, :], ps[:],
                                             AF.Identity,
                                             bias=bq_t[:, m:m + 1],
                                             scale=ISCALE)

                # ============ attention (ow/qT1/out-staging reuse AB space) ==
                with tc.tile_pool(name="ow_w", bufs=1) as owp, \
                     tc.tile_pool(name="qT1_s", bufs=1) as qt1s, \
                     tc.tile_pool(name="obp", bufs=1) as obp, \
                     tc.tile_pool(name="outsb", bufs=4) as osb:
                    # Pool/SWDGE queue again: these generate after the kp
                    # stream + wq/qt0, landing well before first use (~160us+)
                    qt1 = qt1s.tile([P, DT, QB], bf16, name="qt1")
                    nc.gpsimd.dma_start(qt1[:], qT_r[:, :, QB:2 * QB])
                    ow_t = owp.tile([P, ET, D], bf16, name="ow_t")
                    nc.gpsimd.dma_start(ow_t[:, 0:4, :], ow_r[:, 0:4, :])
                    nc.gpsimd.dma_start(ow_t[:, 4:8, :], ow_r[:, 4:8, :])
                    ob_t = obp.tile([P, D], f32, name="ob_t")
                    nc.gpsimd.dma_start(ob_t[:], ob_bc)
                    obsel_t = obp.tile([P, P], bf16, name="obsel_t")
                    nc.gpsimd.dma_start(obsel_t[:], ob_sel)
                    obmat_t = obp.tile([P, D], bf16, name="obmat_t")
                    nc.gpsimd.dma_start(obmat_t[:], ob_mat)

                    def out_group(qb, g, spare_pool, split=False):
                        """out[qb*QB+mq*128 : +128, nd*512 : +512] (8 mm)."""
                        nd, mq = divmod(g, MQ)
                        ps = spare_pool.tile([P, DNB], f32, tag="sp",
                                             name=f"ops{qb}_{g}")
                        for e in range(ET):
                            mm(ps[:], ctxs[qb][:, e, mq * P:(mq + 1) * P],
                               ow_t[:, e, nd * DNB:(nd + 1) * DNB],
                               e == 0, e == ET - 1)
                        r0 = qb * QB + mq * P
                        ot = osb.tile([P, DNB], f32, tag="ot",
                                      name=f"ot{qb}_{g}")
                        nc.vector.tensor_add(
                            ot[:], ps[:], ob_t[:, nd * DNB:(nd + 1) * DNB])
                        nc.gpsimd.dma_start(
                            out[r0:r0 + P, nd * DNB:(nd + 1) * DNB], ot[:])

                    def out_group_tail(qb, g, tail_pool):
                        """Last group: bias folded in via the delta-row
                        matmul, evac by plain ACT copy (no DVE bias add on
                        the critical path), 2 column chunks so chunk 0's
                        store overlaps chunk 1's matmuls; final store rides
                        the fast HWDGE queue."""
                        nd, mq = divmod(g, MQ)
                        r0 = qb * QB + mq * P
                        engs = [nc.gpsimd, nc.sync]
                        for j in range(2):
                            c0 = nd * DNB + j * (DNB // 2)
                            w = DNB // 2
                            ps = tail_pool.tile([P, w], f32, tag=f"tp{j}",
                                                name=f"opst{qb}_{g}_{j}")
                            ot = osb.tile([P, w], f32, tag=f"ott{j}",
                                          name=f"ott{qb}_{g}_{j}")
                            mm(ps[:], obsel_t[:], obmat_t[:, c0:c0 + w],
                               True, False)
                            for e in range(ET):
                                mm(ps[:],
                                   ctxs[qb][:, e, mq * P:(mq + 1) * P],
                                   ow_t[:, e, c0:c0 + w],
                                   False, e == ET - 1)
                            nc.scalar.activation(ot[:], ps[:], AF.Identity)
                            engs[j].dma_start(
                                out[r0:r0 + P, c0:c0 + w], ot[:])

                    for qb in range(NQB):
                        # banks 0-3: ctx half1 accumulators (live whole block)
                        cps_cm = tc.tile_pool(name=f"cps{qb}", bufs=1,
                                              space="PSUM")
                        cpsp = cps_cm.__enter__()
                        cps = [cpsp.tile([P, QB], f32, name=f"c{qb}_{e}")
                               for e in range(ET // 2)]
                        # banks 4-7: logits ping-pong, sum, spare. Open order
                        # matters: the allocator hands the most-recently-freed
                        # banks to the first-opened pool, and the previous
                        # qb's last-freed banks (ctx-half2, evacuated latest)
                        # must NOT land on lg, whose first use is immediate.
                        with tc.tile_pool(name=f"s{qb}", bufs=1,
                                          space="PSUM") as sp, \
                             tc.tile_pool(name=f"spare{qb}", bufs=1,
                                          space="PSUM") as spp, \
                             tc.tile_pool(name=f"lg{qb}", bufs=2,
                                          space="PSUM") as lgp:
                            s_ps = sp.tile([P, QB], f32, name=f"sps{qb}")

                            def lg_mm(kb):
                                ps = lgp.tile([P, QB], f32, tag="lg",
                                              name=f"lg{qb}_{kb}")
                                for e in range(ET):
                                    mm(ps[:], kp[:, e, kb * P:(kb + 1) * P],
                                       qps[qb][:, e, :], e == 0, e == ET - 1)
                                nc.scalar.activation(
                                    expT[:, kb, :], ps[:], AF.Exp,
                                    bias=mask_t[:, kb:kb + 1])

                            def tail_mm(kb):
                                mm(s_ps[:], ones_t[:], expT[:, kb, :],
                                   kb == 0, kb == KT - 1)
                                for e in range(ET // 2):
                                    mm(cps[e][:],
                                       vp[:, kb, e * P:(e + 1) * P],
                                       expT[:, kb, :], kb == 0, kb == KT - 1)

                            for kb in range(KT):
                                lg_mm(kb)
                                if kb > 0:
                                    tail_mm(kb - 1)
                                # spare-bank work, one group per slot:
                                if qb == 0 and kb >= ET:
                                    # qp(qb1) group m = kb-8
                                    m = kb - ET
                                    ps = spp.tile([P, QB], f32, tag="sp",
                                                  name=f"qps1_{m}")
                                    for t in range(DT):
                                        mm(ps[:], wq_t[:, t, m * P:(m + 1) * P],
                                           qt1[:, t, :], t == 0, t == DT - 1)
                                    nc.scalar.activation(
                                        qps[1][:, m, :], ps[:], AF.Identity,
                                        bias=bq_t[:, m:m + 1], scale=ISCALE)
                                if qb == 1 and kb % 2 == 1:
                                    out_group(0, kb // 2, spp)
                            tail_mm(KT - 1)
                            nc.vector.reciprocal(recip_ts[qb][:], s_ps[:])

                        # ctx half2 on a 2-bank ping-pong: only two banks
                        # inherit late evacuations, so the next qb's logits
                        # pool (first-fit) lands on early-freed banks and
                        # starts without waiting
                        with tc.tile_pool(name=f"cps2_{qb}", bufs=2,
                                          space="PSUM") as cps2p:
                            # evacuate half1 (DVE) while half2 accumulates
                            for e in range(ET // 2):
                                nc.vector.tensor_mul(ctxs[qb][:, e, :],
                                                     cps[e][:],
                                                     recip_ts[qb][:])
                            for ei in range(ET // 2):
                                e = ET // 2 + ei
                                c2 = cps2p.tile([P, QB], f32, tag="c2",
                                                name=f"c2_{qb}_{e}")
                                for kb in range(KT):
                                    mm(c2[:],
                                       vp[:, kb, e * P:(e + 1) * P],
                                       expT[:, kb, :], kb == 0, kb == KT - 1)
                                # evac right away so the bank frees for the
                                # next sweep / next qb's pools
                                nc.vector.tensor_mul(ctxs[qb][:, e, :],
                                                     c2[:],
                                                     recip_ts[qb][:])
                        cps_cm.__exit__(None, None, None)

                    # ---- final out phase: out(qb1) ----
                    with tc.tile_pool(name="out_ps", bufs=2,
                                      space="PSUM") as ops, \
                         tc.tile_pool(name="tail_ps", bufs=1,
                                      space="PSUM") as tps:
                        for g in range(ND * MQ - 1):
                            out_group(1, g, ops)
                        out_group_tail(1, ND * MQ - 1, tps)

    nc.compile()
    return nc


def make_in_maps(v, k, q, mask, wq_w, wq_b, wk_w, wk_b, wv_w, wv_b, out_w, out_b,
                 n_cores=8, D=1024, E=1024, SK=2048, QSH=1024):
    """Host-side shard + layout prep (data movement + bf16 cast, no math)."""
    import ml_dtypes
    bf = ml_dtypes.bfloat16
    ET = E // P
    KT = SK // P
    f = np.float32
    wq_w = np.ascontiguousarray(np.asarray(wq_w, f).astype(bf))
    wk_w = np.ascontiguousarray(np.asarray(wk_w, f).astype(bf))
    wv_w = np.ascontiguousarray(np.asarray(wv_w, f).astype(bf))
    out_w = np.ascontiguousarray(np.asarray(out_w, f).astype(bf))
    bq_col = np.ascontiguousarray(np.asarray(wq_b, f).reshape(ET, P).T)
    bk_col = np.ascontiguousarray(np.asarray(wk_b, f).reshape(ET, P).T)
    bv_bc = np.ascontiguousarray(
        np.broadcast_to(np.asarray(wv_b, f).astype(bf), (P, E)))
    ob_bc = np.ascontiguousarray(
        np.broadcast_to(np.asarray(out_b, f), (P, len(out_b))))
    ones_arr = np.ones((P, P), bf)
    ob_sel = np.zeros((P, P), bf)
    ob_sel[0, :] = 1
    ob_mat = np.zeros((P, len(out_b)), bf)
    ob_mat[0, :] = np.asarray(out_b, f).astype(bf)
    in_maps = []
    for c in range(n_cores):
        b, h = divmod(c, 2)
        qTc = np.ascontiguousarray(
            np.asarray(q[b, h * QSH:(h + 1) * QSH, :], f).T.astype(bf))
        kTc = np.ascontiguousarray(np.asarray(k[b], f).T.astype(bf))
        vTc = np.ascontiguousarray(np.asarray(v[b], f).T.astype(bf))
        mc = np.ascontiguousarray(np.asarray(mask[b, 0], f).reshape(KT, P).T)
        in_maps.append(dict(qT=qTc, kT=kTc, vT=vTc, mask_cols=mc,
                            ones_d=ones_arr,
                            wq=wq_w, wk=wk_w, wv=wv_w, ow=out_w,
                            bq_col=bq_col, bk_col=bk_col,
                            bv_bc=bv_bc, ob_bc=ob_bc,
                            ob_sel=ob_sel, ob_mat=ob_mat))
    return in_maps


_NC_CACHE = {}


def kernel(v, k, q, mask, wq_w, wq_b, wk_w, wk_b, wv_w, wv_b, out_w, out_b):
    from concourse.bass_utils import run_bass_kernel_spmd

    B, S, D = 4, 2048, 1024
    E, QSH = 1024, 1024
    if "nc" not in _NC_CACHE:
        _NC_CACHE["nc"] = build_nc(D=D, E=E, SK=S, QSH=QSH, QB=512)
    nc = _NC_CACHE["nc"]

    in_maps = make_in_maps(v, k, q, mask, wq_w, wq_b, wk_w, wk_b, wv_w, wv_b,
                           out_w, out_b, n_cores=8, D=D, E=E, SK=S, QSH=QSH)
    trace = bool(int(os.environ.get("BASS_KERNEL_TRACE", "0")))
    res = run_bass_kernel_spmd(nc, in_maps, core_ids=list(range(8)), trace=trace)
    if trace:
        print(f"HW exec time: {res.exec_time_ns} ns")
        _NC_CACHE["last_exec_time_ns"] = res.exec_time_ns
        _NC_CACHE["last_trace"] = res.instructions_and_trace

    outp = np.empty((B, S, D), np.float32)
    for c in range(8):
        b, h = divmod(c, 2)
        outp[b, h * QSH:(h + 1) * QSH, :] = res.results[c]["out"]
    return outp


# revision 61
# speedup vs baseline: 1.3014x; 1.0012x over previous
"""Single-head attention (B=4, S=2048, D=E=1024) on 8 trn2 NeuronCores.

Sharding: data-parallel over (batch, q-half) -> 8 shards. Each core gets a
1024-row q shard plus the full 2048 keys of its batch; K/V projections are
recomputed on both cores of a batch pair (25% extra flops, zero collectives).

All matmul operands are bf16 (host-cast); PSUM accumulation stays fp32, so
per-value RMS error ~0.1% -- far inside the 2e-2 gate. bf16 runs at the same
1 cycle/row PE rate as fp32r but halves DMA + SBUF, which lets every weight
stay resident (no DRAM bounce) and keeps the PE streaming continuously:

  per-core PE work (cycles @2.4GHz):
    vp 131072 + kp 131072 + qp 65536 + logits 131072 + softmax-sum 16384
    + ctx 131072 + out 65536 = 672k cycles = 280.1us ideal

Schedule: vp -> kp -> qp(qb0) -> qb0 kb-loop [logits|exp|sum|ctx-half1, with
qp(qb1) in the spare PSUM bank] -> ctx-half2 -> qb1 kb-loop [with out(qb0) in
the spare bank] -> ctx-half2 -> out(qb1). PSUM never exceeds 8 banks; weights
for each phase are prefetched during the previous phase via sibling pools.
"""

import os
import numpy as np

P = 128
NEG = -1.0e9


def build_nc(D=1024, E=1024, SK=2048, QSH=1024, QB=512):
    """Build the per-core Bass module (SPMD; same program on all cores)."""
    import concourse.bass as bass
    import concourse.mybir as mybir
    import concourse.tile as tile
    from concourse import bacc

    f32 = mybir.dt.float32
    bf16 = mybir.dt.bfloat16
    AF = mybir.ActivationFunctionType

    DT = D // P          # contraction tiles over model dim
    ET = E // P          # enc tiles
    KT = SK // P         # key tiles
    NQB = QSH // QB      # q blocks (2)
    KNB = 512            # key free-dim block for kp
    DNB = 512            # model free-dim block for out
    MQ = QB // P         # q 128-row groups per block (4)
    ND = D // DNB        # out column chunks (2)
    ISCALE = 1.0 / float(np.sqrt(E))

    nc = bacc.Bacc(trn_type="TRN2")

    # ---- I/O (bf16 operands; f32 biases/mask; f32 output) ----
    qT = nc.dram_tensor("qT", [D, QSH], bf16, kind="ExternalInput")[:, :]
    kT = nc.dram_tensor("kT", [D, SK], bf16, kind="ExternalInput")[:, :]
    vT = nc.dram_tensor("vT", [D, SK], bf16, kind="ExternalInput")[:, :]
    mask_cols = nc.dram_tensor("mask_cols", [P, KT], f32, kind="ExternalInput")[:, :]
    ones_d = nc.dram_tensor("ones_d", [P, P], bf16, kind="ExternalInput")[:, :]
    wq = nc.dram_tensor("wq", [D, E], bf16, kind="ExternalInput")[:, :]
    wk = nc.dram_tensor("wk", [D, E], bf16, kind="ExternalInput")[:, :]
    wv = nc.dram_tensor("wv", [D, E], bf16, kind="ExternalInput")[:, :]
    ow = nc.dram_tensor("ow", [E, D], bf16, kind="ExternalInput")[:, :]
    bq_col = nc.dram_tensor("bq_col", [P, ET], f32, kind="ExternalInput")[:, :]
    bk_col = nc.dram_tensor("bk_col", [P, ET], f32, kind="ExternalInput")[:, :]
    bv_bc = nc.dram_tensor("bv_bc", [P, E], bf16, kind="ExternalInput")[:, :]
    ob_bc = nc.dram_tensor("ob_bc", [P, D], f32, kind="ExternalInput")[:, :]
    # delta-row selector + ob with row 0 = out bias: the final out group
    # folds its bias in via matmul so the evac is a plain ACT copy
    ob_sel = nc.dram_tensor("ob_sel", [P, P], bf16, kind="ExternalInput")[:, :]
    ob_mat = nc.dram_tensor("ob_mat", [P, D], bf16, kind="ExternalInput")[:, :]
    out = nc.dram_tensor("out", [QSH, D], f32, kind="ExternalOutput")[:, :]

    qT_r = qT.rearrange("(t p) n -> p t n", p=P)   # [128, DT, QSH]
    kT_r = kT.rearrange("(t p) n -> p t n", p=P)
    vT_r = vT.rearrange("(t p) n -> p t n", p=P)
    wq_r = wq.rearrange("(t p) n -> p t n", p=P)   # [128, DT, E]
    wk_r = wk.rearrange("(t p) n -> p t n", p=P)
    wv_r = wv.rearrange("(t p) n -> p t n", p=P)
    ow_r = ow.rearrange("(t p) n -> p t n", p=P)   # [128, ET, D]

    def mm(ps, lhsT, rhs, start, stop):
        nc.tensor.matmul(ps, lhsT, rhs, start=start, stop=stop)

    NWARM = 10

    with tile.TileContext(nc) as tc:
        # ---- persistent smalls (tiles allocated here; DMAs emitted inside
        # the AB scope so the scalar queue prioritizes wv chunks) ----
        with tc.tile_pool(name="smalls", bufs=1) as smalls:
            bv_t = smalls.tile([P, E], bf16, name="bv_t")
            mask_t = smalls.tile([P, KT], f32, name="maskc")
            bk_t = smalls.tile([P, ET], f32, name="bkc")
            bq_t = smalls.tile([P, ET], f32, name="bqc")
            ones_t = smalls.tile([P, P], bf16, name="ones")
            recip_ts = [smalls.tile([P, QB], f32, name=f"recip{i}")
                        for i in range(NQB)]

            # persistent operand tensors
            with tc.tile_pool(name="wqp", bufs=1) as wqp, \
                 tc.tile_pool(name="vpp", bufs=1) as vpp, \
                 tc.tile_pool(name="kpp", bufs=1) as kpp, \
                 tc.tile_pool(name="qpp", bufs=1) as qpp, \
                 tc.tile_pool(name="expp", bufs=1) as expp, \
                 tc.tile_pool(name="ctxp", bufs=1) as ctxp:
                wq_t = wqp.tile([P, DT, E], bf16, name="wq_t")
                vp = vpp.tile([P, KT, E], bf16, name="vp")      # [k, E]
                kp = kpp.tile([P, ET, SK], bf16, name="kp")     # [E, k] (kp^T)
                qps = [qpp.tile([P, ET, QB], bf16, name=f"qp{i}")
                       for i in range(NQB)]                      # [E, q] (qp^T)
                expT = expp.tile([P, KT, QB], bf16, name="expT")  # [k, q]
                ctxs = [ctxp.tile([P, ET, QB], bf16, name=f"ctx{i}")
                        for i in range(NQB)]                     # [E, q] (ctx^T)

                # ============ phase A+B: vp then kp (sibling pools so kp
                # weights prefetch during vp) ============
                with tc.tile_pool(name="wv_w", bufs=1) as wvp, \
                     tc.tile_pool(name="wk_w", bufs=1) as wkp, \
                     tc.tile_pool(name="vT_s", bufs=2) as vts, \
                     tc.tile_pool(name="kT_s", bufs=2) as kts, \
                     tc.tile_pool(name="qT0_s", bufs=1) as qt0s, \
                     tc.tile_pool(name="warm", bufs=1) as warm, \
                     tc.tile_pool(name="warm_ps", bufs=1,
                                  space="PSUM") as wps, \
                     tc.tile_pool(name="ab_ps", bufs=3, space="PSUM") as abps:
                    # PE warm-up: dummy matmuls on a memset tile fill the
                    # ~5.7us wait for the first weight DMA and ramp the PE
                    # p-state so real work starts at full clock. Lives in
                    # the AB scope so nothing aliases (and WAR-waits on) it.
                    junk = warm.tile([P, 512], bf16, name="junk")
                    nc.vector.memset(junk[:], 0.0)
                    wp = wps.tile([P, 512], f32, name="warmps")
                    for i in range(NWARM):
                        nc.tensor.matmul(wp[:], junk[:, 0:P], junk[:],
                                         start=(i == 0),
                                         stop=(i == NWARM - 1))
                    # The shared DMA bus serves transfers in descriptor-gen
                    # completion order, so every queue is sequenced by first
                    # NEED: sync gets only the first wv half; Pool (slow 1.3us
                    # SWDGE gen each = natural pacing) carries the whole
                    # vp/kp-phase stream in consumption order; scalar gets the
                    # smalls then the late-needed wq/qt0.
                    wv_t = wvp.tile([P, DT, E], bf16, name="wv_t")
                    wk_t = wkp.tile([P, DT, E], bf16, name="wk_t")
                    nc.sync.dma_start(wv_t[:, 0:4, 0:512], wv_r[:, 0:4, 0:512])
                    nc.scalar.dma_start(wv_t[:, 4:8, 0:512],
                                        wv_r[:, 4:8, 0:512])
                    # smalls on scalar, ordered by first use (both bv halves
                    # ride the Pool queue so they can't delay the vT stream)
                    nc.scalar.dma_start(mask_t[:], mask_cols)
                    nc.scalar.mul(mask_t[:], mask_t[:], NEG)
                    nc.scalar.dma_start(bk_t[:], bk_col)
                    nc.scalar.dma_start(bq_t[:], bq_col)
                    nc.scalar.mul(bq_t[:], bq_t[:], ISCALE)
                    nc.scalar.dma_start(ones_t[:], ones_d)
                    qt0 = qt0s.tile([P, DT, QB], bf16, name="qt0")

                    # -- vp: psum [128k, 512E] per (m, n) group; vT streamed
                    # in 1MB chunks of 4 k-tiles (desc count is per (p,t),
                    # so wider chunks halve Pool SWDGE time) --
                    vtiles = {}

                    def load_vt(c):
                        vt = vts.tile([P, DT, 4 * P], bf16, tag="vt",
                                      name=f"vt{c}")
                        if c == 0:
                            # split so each early k-tile unblocks as it lands
                            nc.gpsimd.dma_start(vt[:, :, 0:P],
                                                vT_r[:, :, 0:P])
                            nc.gpsimd.dma_start(vt[:, :, P:2 * P],
                                                vT_r[:, :, P:2 * P])
                            nc.gpsimd.dma_start(vt[:, :, 2 * P:4 * P],
                                                vT_r[:, :, 2 * P:4 * P])
                        else:
                            nc.gpsimd.dma_start(
                                vt[:], vT_r[:, :, 4 * c * P:(4 * c + 4) * P])
                        vtiles[c] = vt

                    def vp_group(m, n):
                        ps = abps.tile([P, 512], f32, tag="ps",
                                       name=f"vps{m}_{n}")
                        vt = vtiles[m // 4]
                        mi = m % 4
                        for t in range(DT):
                            mm(ps[:], vt[:, t, mi * P:(mi + 1) * P],
                               wv_t[:, t, n * 512:(n + 1) * 512],
                               t == 0, t == DT - 1)
                        nc.vector.tensor_add(
                            vp[:, m, n * 512:(n + 1) * 512], ps[:],
                            bv_t[:, n * 512:(n + 1) * 512])

                    # chunk 0: n=0 groups first (wv col half 1 still loading)
                    load_vt(0)
                    # Pool/SWDGE queue carries everything else in strict
                    # consumption order (scalar/sync are HWDGE-fast and would
                    # let late-needed weights jump the shared bus)
                    nc.gpsimd.dma_start(bv_t[:, 0:512], bv_bc[:, 0:512])
                    nc.gpsimd.dma_start(wv_t[:, :, 512:1024],
                                        wv_r[:, :, 512:1024])
                    nc.gpsimd.dma_start(bv_t[:, 512:1024], bv_bc[:, 512:1024])
                    load_vt(1)
                    for m in range(4):
                        vp_group(m, 0)
                    for m in range(4):
                        vp_group(m, 1)
                    for c in range(1, KT // 4):
                        if c + 1 < KT // 4:
                            load_vt(c + 1)
                        for mi in range(4):
                            for n in range(E // 512):
                                vp_group(4 * c + mi, n)
                    # wk lands on the bus after vt2/vt3 (needed at kp start)
                    nc.gpsimd.dma_start(wk_t[:, :, 0:512], wk_r[:, :, 0:512])
                    nc.gpsimd.dma_start(wk_t[:, :, 512:1024],
                                        wk_r[:, :, 512:1024])

                    # -- kp: for each k-chunk, psum [128E, 512k] x8 --
                    for n in range(SK // KNB):
                        kt = kts.tile([P, DT, KNB], bf16, tag="kt",
                                      name=f"kt{n}")
                        nc.gpsimd.dma_start(kt[:],
                                            kT_r[:, :, n * KNB:(n + 1) * KNB])
                        for m in range(ET):
                            ps = abps.tile([P, KNB], f32, tag="ps",
                                           name=f"kps{n}_{m}")
                            for t in range(DT):
                                mm(ps[:], wk_t[:, t, m * P:(m + 1) * P],
                                   kt[:, t, :], t == 0, t == DT - 1)
                            nc.scalar.activation(
                                kp[:, m, n * KNB:(n + 1) * KNB], ps[:],
                                AF.Identity, bias=bk_t[:, m:m + 1])

                    # wq/qt0 queue behind the kp stream; kt2/kt3's WAR head-
                    # of-line block paces their generation to ~90us, arriving
                    # in time for qp0 at ~116us
                    nc.gpsimd.dma_start(wq_t[:, 0:4, :], wq_r[:, 0:4, :])
                    nc.gpsimd.dma_start(wq_t[:, 4:8, :], wq_r[:, 4:8, :])
                    nc.gpsimd.dma_start(qt0[:], qT_r[:, :, 0:QB])

                    # -- qp(qb0): psum [128E, 512q] x8 (reuse ab psum bufs) --
                    for m in range(ET):
                        ps = abps.tile([P, QB], f32, tag="ps", name=f"qps0_{m}")
                        for t in range(DT):
                            mm(ps[:], wq_t[:, t, m * P:(m + 1) * P],
                               qt0[:, t, :], t == 0, t == DT - 1)
                        nc.scalar.activation(qps[0][:, m, :], ps[:],
                                             AF.Identity,
                                             bias=bq_t[:, m:m + 1],
                                             scale=ISCALE)

                # ============ attention (ow/qT1/out-staging reuse AB space) ==
                with tc.tile_pool(name="ow_w", bufs=1) as owp, \
                     tc.tile_pool(name="qT1_s", bufs=1) as qt1s, \
                     tc.tile_pool(name="obp", bufs=1) as obp, \
                     tc.tile_pool(name="outsb", bufs=4) as osb:
                    # Pool/SWDGE queue again: these generate after the kp
                    # stream + wq/qt0, landing well before first use (~160us+)
                    qt1 = qt1s.tile([P, DT, QB], bf16, name="qt1")
                    nc.gpsimd.dma_start(qt1[:], qT_r[:, :, QB:2 * QB])
                    ow_t = owp.tile([P, ET, D], bf16, name="ow_t")
                    nc.gpsimd.dma_start(ow_t[:, 0:4, :], ow_r[:, 0:4, :])
                    nc.gpsimd.dma_start(ow_t[:, 4:8, :], ow_r[:, 4:8, :])
                    ob_t = obp.tile([P, D], f32, name="ob_t")
                    nc.gpsimd.dma_start(ob_t[:], ob_bc)
                    obsel_t = obp.tile([P, P], bf16, name="obsel_t")
                    nc.gpsimd.dma_start(obsel_t[:], ob_sel)
                    obmat_t = obp.tile([P, D], bf16, name="obmat_t")
                    nc.gpsimd.dma_start(obmat_t[:], ob_mat)

                    def out_group(qb, g, spare_pool, split=False):
                        """out[qb*QB+mq*128 : +128, nd*512 : +512] (8 mm)."""
                        nd, mq = divmod(g, MQ)
                        ps = spare_pool.tile([P, DNB], f32, tag="sp",
                                             name=f"ops{qb}_{g}")
                        for e in range(ET):
                            mm(ps[:], ctxs[qb][:, e, mq * P:(mq + 1) * P],
                               ow_t[:, e, nd * DNB:(nd + 1) * DNB],
                               e == 0, e == ET - 1)
                        r0 = qb * QB + mq * P
                        ot = osb.tile([P, DNB], f32, tag="ot",
                                      name=f"ot{qb}_{g}")
                        nc.vector.tensor_add(
                            ot[:], ps[:], ob_t[:, nd * DNB:(nd + 1) * DNB])
                        nc.gpsimd.dma_start(
                            out[r0:r0 + P, nd * DNB:(nd + 1) * DNB], ot[:])

                    def out_group_tail(qb, g, tail_pool):
                        """Last group: bias folded in via the delta-row
                        matmul, evac by plain ACT copy (no DVE bias add on
                        the critical path), 2 column chunks so chunk 0's
                        store overlaps chunk 1's matmuls; final store rides
                        the fast HWDGE queue."""
                        nd, mq = divmod(g, MQ)
                        r0 = qb * QB + mq * P
                        engs = [nc.gpsimd, nc.sync]
                        widths = [DNB // 2, DNB // 2]
                        for j in range(2):
                            c0 = nd * DNB + j * widths[0]
                            w = widths[j]
                            ps = tail_pool.tile([P, w], f32, tag=f"tp{j}",
                                                name=f"opst{qb}_{g}_{j}")
                            ot = osb.tile([P, w], f32, tag=f"ott{j}",
                                          name=f"ott{qb}_{g}_{j}")
                            mm(ps[:], obsel_t[:], obmat_t[:, c0:c0 + w],
                               True, False)
                            for e in range(ET):
                                mm(ps[:],
                                   ctxs[qb][:, e, mq * P:(mq + 1) * P],
                                   ow_t[:, e, c0:c0 + w],
                                   False, e == ET - 1)
                            nc.scalar.activation(ot[:], ps[:], AF.Identity)
                            engs[j].dma_start(
                                out[r0:r0 + P, c0:c0 + w], ot[:])

                    for qb in range(NQB):
                        # banks 0-3: ctx half1 accumulators (live whole block)
                        cps_cm = tc.tile_pool(name=f"cps{qb}", bufs=1,
                                              space="PSUM")
                        cpsp = cps_cm.__enter__()
                        cps = [cpsp.tile([P, QB], f32, name=f"c{qb}_{e}")
                               for e in range(ET // 2)]
                        # banks 4-7: logits ping-pong, sum, spare. Open order
                        # matters: the allocator hands the most-recently-freed
                        # banks to the first-opened pool, and the previous
                        # qb's last-freed banks (ctx-half2, evacuated latest)
                        # must NOT land on lg, whose first use is immediate.
                        with tc.tile_pool(name=f"s{qb}", bufs=1,
                                          space="PSUM") as sp, \
                             tc.tile_pool(name=f"spare{qb}", bufs=1,
                                          space="PSUM") as spp, \
                             tc.tile_pool(name=f"lg{qb}", bufs=2,
                                          space="PSUM") as lgp:
                            s_ps = sp.tile([P, QB], f32, name=f"sps{qb}")

                            def lg_mm(kb):
                                ps = lgp.tile([P, QB], f32, tag="lg",
                                              name=f"lg{qb}_{kb}")
                                for e in range(ET):
                                    mm(ps[:], kp[:, e, kb * P:(kb + 1) * P],
                                       qps[qb][:, e, :], e == 0, e == ET - 1)
                                nc.scalar.activation(
                                    expT[:, kb, :], ps[:], AF.Exp,
                                    bias=mask_t[:, kb:kb + 1])

                            def tail_mm(kb):
                                mm(s_ps[:], ones_t[:], expT[:, kb, :],
                                   kb == 0, kb == KT - 1)
                                for e in range(ET // 2):
                                    mm(cps[e][:],
                                       vp[:, kb, e * P:(e + 1) * P],
                                       expT[:, kb, :], kb == 0, kb == KT - 1)

                            for kb in range(KT):
                                lg_mm(kb)
                                if kb > 0:
                                    tail_mm(kb - 1)
                                # spare-bank work, one group per slot:
                                if qb == 0 and kb >= ET:
                                    # qp(qb1) group m = kb-8
                                    m = kb - ET
                                    ps = spp.tile([P, QB], f32, tag="sp",
                                                  name=f"qps1_{m}")
                                    for t in range(DT):
                                        mm(ps[:], wq_t[:, t, m * P:(m + 1) * P],
                                           qt1[:, t, :], t == 0, t == DT - 1)
                                    nc.scalar.activation(
                                        qps[1][:, m, :], ps[:], AF.Identity,
                                        bias=bq_t[:, m:m + 1], scale=ISCALE)
                                if qb == 1 and kb % 2 == 1:
                                    out_group(0, kb // 2, spp)
                            tail_mm(KT - 1)
                            nc.vector.reciprocal(recip_ts[qb][:], s_ps[:])

                        # ctx half2 on a 2-bank ping-pong: only two banks
                        # inherit late evacuations, so the next qb's logits
                        # pool (first-fit) lands on early-freed banks and
                        # starts without waiting
                        with tc.tile_pool(name=f"cps2_{qb}", bufs=2,
                                          space="PSUM") as cps2p:
                            # evacuate half1 (DVE) while half2 accumulates
                            for e in range(ET // 2):
                                nc.vector.tensor_mul(ctxs[qb][:, e, :],
                                                     cps[e][:],
                                                     recip_ts[qb][:])
                            for ei in range(ET // 2):
                                e = ET // 2 + ei
                                c2 = cps2p.tile([P, QB], f32, tag="c2",
                                                name=f"c2_{qb}_{e}")
                                for kb in range(KT):
                                    mm(c2[:],
                                       vp[:, kb, e * P:(e + 1) * P],
                                       expT[:, kb, :], kb == 0, kb == KT - 1)
                                # evac right away so the bank frees for the
                                # next sweep / next qb's pools
                                nc.vector.tensor_mul(ctxs[qb][:, e, :],
                                                     c2[:],
                                                     recip_ts[qb][:])
                        cps_cm.__exit__(None, None, None)

                    # ---- final out phase: out(qb1) ----
                    with tc.tile_pool(name="out_ps", bufs=2,
                                      space="PSUM") as ops, \
                         tc.tile_pool(name="tail_ps", bufs=1,
                                      space="PSUM") as tps:
                        for g in range(ND * MQ - 1):
                            out_group(1, g, ops)
                        out_group_tail(1, ND * MQ - 1, tps)

    nc.compile()
    return nc


def make_in_maps(v, k, q, mask, wq_w, wq_b, wk_w, wk_b, wv_w, wv_b, out_w, out_b,
                 n_cores=8, D=1024, E=1024, SK=2048, QSH=1024):
    """Host-side shard + layout prep (data movement + bf16 cast, no math)."""
    import ml_dtypes
    bf = ml_dtypes.bfloat16
    ET = E // P
    KT = SK // P
    f = np.float32
    wq_w = np.ascontiguousarray(np.asarray(wq_w, f).astype(bf))
    wk_w = np.ascontiguousarray(np.asarray(wk_w, f).astype(bf))
    wv_w = np.ascontiguousarray(np.asarray(wv_w, f).astype(bf))
    out_w = np.ascontiguousarray(np.asarray(out_w, f).astype(bf))
    bq_col = np.ascontiguousarray(np.asarray(wq_b, f).reshape(ET, P).T)
    bk_col = np.ascontiguousarray(np.asarray(wk_b, f).reshape(ET, P).T)
    bv_bc = np.ascontiguousarray(
        np.broadcast_to(np.asarray(wv_b, f).astype(bf), (P, E)))
    ob_bc = np.ascontiguousarray(
        np.broadcast_to(np.asarray(out_b, f), (P, len(out_b))))
    ones_arr = np.ones((P, P), bf)
    ob_sel = np.zeros((P, P), bf)
    ob_sel[0, :] = 1
    ob_mat = np.zeros((P, len(out_b)), bf)
    ob_mat[0, :] = np.asarray(out_b, f).astype(bf)
    in_maps = []
    for c in range(n_cores):
        b, h = divmod(c, 2)
        qTc = np.ascontiguousarray(
            np.asarray(q[b, h * QSH:(h + 1) * QSH, :], f).T.astype(bf))
        kTc = np.ascontiguousarray(np.asarray(k[b], f).T.astype(bf))
        vTc = np.ascontiguousarray(np.asarray(v[b], f).T.astype(bf))
        mc = np.ascontiguousarray(np.asarray(mask[b, 0], f).reshape(KT, P).T)
        in_maps.append(dict(qT=qTc, kT=kTc, vT=vTc, mask_cols=mc,
                            ones_d=ones_arr,
                            wq=wq_w, wk=wk_w, wv=wv_w, ow=out_w,
                            bq_col=bq_col, bk_col=bk_col,
                            bv_bc=bv_bc, ob_bc=ob_bc,
                            ob_sel=ob_sel, ob_mat=ob_mat))
    return in_maps


_NC_CACHE = {}


def kernel(v, k, q, mask, wq_w, wq_b, wk_w, wk_b, wv_w, wv_b, out_w, out_b):
    from concourse.bass_utils import run_bass_kernel_spmd

    B, S, D = 4, 2048, 1024
    E, QSH = 1024, 1024
    if "nc" not in _NC_CACHE:
        _NC_CACHE["nc"] = build_nc(D=D, E=E, SK=S, QSH=QSH, QB=512)
    nc = _NC_CACHE["nc"]

    in_maps = make_in_maps(v, k, q, mask, wq_w, wq_b, wk_w, wk_b, wv_w, wv_b,
                           out_w, out_b, n_cores=8, D=D, E=E, SK=S, QSH=QSH)
    trace = bool(int(os.environ.get("BASS_KERNEL_TRACE", "0")))
    res = run_bass_kernel_spmd(nc, in_maps, core_ids=list(range(8)), trace=trace)
    if trace:
        print(f"HW exec time: {res.exec_time_ns} ns")
        _NC_CACHE["last_exec_time_ns"] = res.exec_time_ns
        _NC_CACHE["last_trace"] = res.instructions_and_trace

    outp = np.empty((B, S, D), np.float32)
    for c in range(8):
        b, h = divmod(c, 2)
        outp[b, h * QSH:(h + 1) * QSH, :] = res.results[c]["out"]
    return outp
